# revision 27
# baseline (speedup 1.0000x reference)
"""Trainium2 Bass kernel for nn_EquivariantProductBasisBlock (MACE product basis).

Per (node b, channel c) the block computes a symmetric cubic polynomial in
x = node_feats[b,c,:] (16-dim). v2 basis: powers of linear forms.

  quad monomials q_ij = x_i x_j   -> spanned by squares   (a2*(x_i+x_j))^2
  cubic monomials t_ijm           -> spanned by cubes     (a3*(x_i+x_j+x_m))^3

The change of basis (A2/A3, cond ~6/21) is folded into U on the host in fp64,
so on-chip each basis tile is: one PE selection matmul (ell = Sel @ x), one
ScalarE Square (PSUM->SBUF), and for cubics one DVE/Pool scalar_tensor_tensor
(ell * ell^2). Basis tiles are paired into [128,1024] two-bank PSUM supertiles
so one ScalarE Square covers two tiles.

  G[(ld,kap), n] = Ufold.T @ [x; squares; cubes]      -- 10 PE matmuls / block
  Wrep[c,(kap,b)] = WK.T @ attrs.T (bf16 PE)          -- exact for dense attrs
  out1[c,(b,ld)]  = sum_kap G'[c,(b,ld,kap)] * Wrep'  -- transpose + VE
  out[b] = concat_li(lin_li.T @ out1)/sqrt(C) + sc

Sharding: data-parallel over nodes, 128 nodes/core on 8 cores, no collectives.
"""
import math
import os
import numpy as np

N, C, L, E = 1024, 128, 16, 10
NCORES = 8
BLOC = N // NCORES            # nodes per core
NLOC = BLOC * C               # (b,c) columns per core; n = b*C + c
NB = 512                      # column block (one fp32 PSUM bank)
NBLK = NLOC // NB
NNOD = NB // C                # nodes per block
LBLK = (NBLK + 2) // 3        # column blocks per partition lane (X packing)
LANEW = LBLK * NB             # free width per lane

PAIRS = [(i, j) for j in range(L) for i in range(j + 1)]              # 136
TRIPLES = [(i, j, m) for j in range(L) for i in range(j + 1) for m in range(j, L)]
NQ, NT = len(PAIRS), len(TRIPLES)                                      # 136, 816

# 7 cubic basis tiles of <=128 rows (base partition 0, zero-padded).  The 136
# "special" triples (i,j,15) live in tiles 5/6; their Act-Square intermediates
# (ell^2) span all quadratic monomials, so the quad U-path folds into two
# extra G matmuls reading c2 of tiles 5/6 — no dedicated quadratic tiles.
NTILE = 7
NSLOT = 9                     # U_all slots: 7 cube + 2 quad (c2 of tiles 5,6)

A2S = 1.0 / math.sqrt(2.0)    # scale for quad linear forms
A3S = 1.0 / math.sqrt(3.0)    # scale for cubic linear forms


def _build_consts(inputs):
    import itertools
    f32 = np.float32
    Us = [{nu: np.asarray(inputs[f"U_{li}_{nu}"], np.float64) for nu in (1, 2, 3)}
          for li in range(2)]
    lins = [np.asarray(inputs[f"lin_{li}"], f32) for li in range(2)]

    row_of_pair = {p: r for r, p in enumerate(PAIRS)}
    row_of_triple = {}
    for r, (i, j, m) in enumerate(TRIPLES):
        row_of_triple[tuple(sorted((i, j, m)))] = r

    # base U coefficients on monomial bases (as in the reference contraction)
    UX = np.zeros((16, 64), np.float64)
    Uq = np.zeros((NQ, 64), np.float64)
    U3 = np.zeros((NT, 64), np.float64)
    for ld in range(4):
        li, dd = (0, 0) if ld == 0 else (1, ld - 1)
        U3t, U2t, U1t = Us[li][3], Us[li][2], Us[li][1]
        UX[:, ld * 16 + 15] = U1t[dd, :, 0]
        for r, (i, j) in enumerate(PAIRS):
            v = U2t[dd, i, j, :] + (U2t[dd, j, i, :] if i != j else 0.0)
            Uq[r, ld * 16 + 11:ld * 16 + 15] = v
        for r, (i, j, m) in enumerate(TRIPLES):
            if i < j < m:
                arr = [(i, j, m), (i, m, j), (j, i, m), (j, m, i), (m, i, j), (m, j, i)]
            elif i == j and j < m:
                arr = [(i, i, m), (i, m, i), (m, i, i)]
            elif i < j and j == m:
                arr = [(i, j, j), (j, i, j), (j, j, i)]
            else:
                arr = [(i, i, i)]
            U3[r, ld * 16:ld * 16 + 11] = sum(U3t[dd, a, b, c, :] for (a, b, c) in arr)

    # cubic change of basis: y3 = A3 t  (y3_r = (a3(x_i+x_j+x_m))^3)
    A3 = np.zeros((NT, NT))
    for r, (i, j, m) in enumerate(TRIPLES):
        for (u, v, w) in itertools.product((i, j, m), repeat=3):
            A3[r, row_of_triple[tuple(sorted((u, v, w)))]] += 1.0
    U3f = np.linalg.solve(A3.T * (A3S ** 3), U3)

    # quad fold: q monomials via (a3(x_i+x_j+x_15))^2 of the special triples
    B = np.zeros((NQ, NQ))
    for r, (i, j) in enumerate(PAIRS):
        cv = np.zeros(16)
        cv[i] += A3S; cv[j] += A3S; cv[15] += A3S
        for a in range(16):
            for b in range(a, 16):
                coef = cv[a] * cv[b] * (2.0 if a != b else 1.0)
                if coef:
                    B[r, row_of_pair[(a, b)]] += coef
    Vq = np.linalg.solve(B.T, Uq)                    # [136, 64] on special forms

    # triple ordering: OTHER (m != 15) first, then SPECIAL = (i,j,15) in PAIRS
    # order.  Tiles: 0..4 = other[128k:128(k+1)], 5 = other[640:680]+special[0:88],
    # 6 = special[88:136] (zero-padded)
    special_orig = [row_of_triple[tuple(sorted((i, j, 15)))] for (i, j) in PAIRS]
    other_orig = [r for r, t in enumerate(TRIPLES) if t[2] != 15]
    assert len(other_orig) == NT - NQ

    def tile_row(ti, p):
        """('o'|'s', idx) of the basis row at partition p of tile ti, or None"""
        if ti < 5:
            return ("o", ti * 128 + p)
        if ti == 5:
            return ("o", 640 + p) if p < 40 else ("s", p - 40)
        return ("s", 88 + p) if p < 48 else None

    SEL = np.zeros((16, NTILE * 128), np.float64)
    U_all = np.zeros((128, 64 * NSLOT), np.float64)
    for ti in range(NTILE):
        for p in range(128):
            r = tile_row(ti, p)
            if r is None:
                continue
            kind, k = r
            orig = other_orig[k] if kind == "o" else special_orig[k]
            i, j, m = TRIPLES[orig]
            SEL[i, ti * 128 + p] += A3S
            SEL[j, ti * 128 + p] += A3S
            SEL[m, ti * 128 + p] += A3S
            U_all[p, ti * 64:(ti + 1) * 64] = U3f[orig]
            if kind == "s":
                U_all[p, (7 + (ti - 5)) * 64:(8 + (ti - 5)) * 64] = Vq[k]

    # 3-lane packing at partition bases {0,32,64} (lhsT.base == rhs.base)
    def lane3(mat):
        rows = mat.shape[0]
        out = np.zeros((64 + rows, mat.shape[1]), mat.dtype)
        for Lb in range(3):
            out[32 * Lb:32 * Lb + rows] = mat
        return out

    # WK packed at 3 bases: kappa groups 0..23 | 24..47 | 48..63
    Ws = [{nu: np.asarray(inputs[f"W_{li}_{nu}"], f32) for nu in (1, 2, 3)}
          for li in range(2)]
    WKp = np.zeros((E, 64, C), f32)
    for ld in range(4):
        li = 0 if ld == 0 else 1
        WKp[:, ld * 16:ld * 16 + 11, :] = Ws[li][3]
        WKp[:, ld * 16 + 11:ld * 16 + 15, :] = Ws[li][2]
        WKp[:, ld * 16 + 15, :] = Ws[li][1][:, 0, :]
    WK3 = np.zeros((74, 24 * C), f32)
    for kap in range(64):
        g, off = (0, 0) if kap < 24 else ((1, 24) if kap < 48 else (2, 48))
        WK3[32 * g:32 * g + E, (kap - off) * C:(kap - off + 1) * C] = WKp[:, kap, :]

    isc = f32(1.0 / math.sqrt(C))
    return {
        "U_all": U_all.astype(f32),
        "UX3": lane3(UX.astype(f32)),
        "SEL3": lane3(SEL.astype(f32)),
        "WK3": WK3,
        "lin0": np.ascontiguousarray(lins[0] * isc),
        "lin1": np.ascontiguousarray(lins[1] * isc),
    }


def build_program():
    import concourse.bass as bass
    import concourse.bacc as bacc
    import concourse.mybir as mybir
    import concourse.tile as tile
    from concourse.masks import make_identity
    from contextlib import ExitStack

    dt = mybir.dt
    F32 = dt.float32
    F32R = dt.float32r
    BF16 = dt.bfloat16
    AX = mybir.AxisListType
    SQUARE = mybir.ActivationFunctionType.Square
    MULT = mybir.AluOpType.mult

    nc = bacc.Bacc(None, target_bir_lowering=False)
    X_Tm = nc.dram_tensor("X_Tm", [80, LANEW], F32, kind="ExternalInput")
    attrsT = nc.dram_tensor("attrsT", [E, BLOC], F32, kind="ExternalInput")
    sc_d = nc.dram_tensor("sc", [BLOC, 512], F32, kind="ExternalInput")
    U_all = nc.dram_tensor("U_all", [128, 64 * NSLOT], F32, kind="ExternalInput")
    UX3 = nc.dram_tensor("UX3", [80, 64], F32, kind="ExternalInput")
    SEL3 = nc.dram_tensor("SEL3", [80, NTILE * 128], F32, kind="ExternalInput")
    WK3 = nc.dram_tensor("WK3", [74, 24 * C], F32, kind="ExternalInput")
    lin0 = nc.dram_tensor("lin0", [C, C], F32, kind="ExternalInput")
    lin1 = nc.dram_tensor("lin1", [C, C], F32, kind="ExternalInput")
    OUT = nc.dram_tensor("OUT", [BLOC, 512], F32, kind="ExternalOutput")

    with tile.TileContext(nc) as tc, ExitStack() as ctx:
        cpool = ctx.enter_context(tc.tile_pool(name="consts", bufs=1))
        fpool = ctx.enter_context(tc.tile_pool(name="feats", bufs=2))
        spool = ctx.enter_context(tc.tile_pool(name="stream", bufs=8))
        dpool = ctx.enter_context(tc.tile_pool(name="dmab", bufs=4))
        # PSUM (8 banks): pair supertile (2 banks x 3 bufs) + g (1x1) + misc (1x1)
        pp_pair = ctx.enter_context(tc.tile_pool(name="ps_pair", bufs=3, space="PSUM"))
        pp_g = ctx.enter_context(tc.tile_pool(name="ps_g", bufs=1, space="PSUM"))
        pp_misc = ctx.enter_context(tc.tile_pool(name="ps_misc", bufs=1, space="PSUM"))

        # PE-consumed tiles laundered through one copy each so matmul operand
        # producers collapse onto a single engine.  Copies are spread across
        # DVE and ScalarE so startup laundering parallelizes.
        def launder(shape, dtp, tag, src, eng=None):
            raw = cpool.tile(shape, src.dtype, tag=tag + "_r")
            nc.sync.dma_start(raw[:], src[:])
            t = cpool.tile(shape, dtp, tag=tag)
            if eng is nc.scalar:
                nc.scalar.copy(t[:], raw[:])
            else:
                (eng or nc.vector).tensor_copy(t[:], raw[:])
            return t

        # attrs + WK first: the wrep build depends only on these
        ats_raw = cpool.tile([74, BLOC], F32, tag="attrs_r")
        ats = cpool.tile([74, BLOC], BF16, tag="attrs")
        for Lb in range(3):
            nc.sync.dma_start(ats_raw[32 * Lb:32 * Lb + E], attrsT[:])
            nc.vector.tensor_copy(ats[32 * Lb:32 * Lb + E], ats_raw[32 * Lb:32 * Lb + E])
        wk3 = launder([74, 24 * C], BF16, "wk3", WK3)
        xsm = launder([80, LANEW], F32R, "xTm", X_Tm, eng=nc.scalar)
        sel3 = launder([80, NTILE * 128], F32R, "sel3", SEL3)
        ua = launder([128, 64 * NSLOT], F32R, "uall", U_all)
        ux3 = launder([80, 64], F32R, "ux3", UX3)
        l0 = launder([C, C], F32, "lin0", lin0)
        l1 = launder([C, C], F32, "lin1", lin1)
        sct = cpool.tile([BLOC, 512], F32, tag="sc"); nc.sync.dma_start(sct[:], sc_d[:])
        ident_raw = cpool.tile([128, 128], F32, tag="ident_r")
        make_identity(nc, ident_raw[:])
        ident = cpool.tile([128, 128], F32, tag="ident")
        nc.vector.tensor_copy(ident[:], ident_raw[:])

        # Wrep' [c, (kap, b)]: 16 rounds x 4 kappa, K=10 bf16 matmuls
        wrep = cpool.tile([C, 64 * BLOC], F32, tag="wrep")
        for rnd in range(16):
            wps = pp_misc.tile([C, 4 * BLOC], F32, tag="misc")
            for kk in range(4):
                kap = rnd * 4 + kk
                g3, off = (0, 0) if kap < 24 else ((1, 24) if kap < 48 else (2, 48))
                nc.tensor.matmul(
                    wps[:, kk * BLOC:(kk + 1) * BLOC],
                    wk3[32 * g3:32 * g3 + E, (kap - off) * C:(kap - off + 1) * C],
                    ats[32 * g3:32 * g3 + E], start=True, stop=True)
            nc.scalar.copy(wrep[:, rnd * 4 * BLOC:(rnd + 1) * 4 * BLOC], wps[:])

        out1 = cpool.tile([C, BLOC * 4], F32, tag="out1")  # [c, (b, ld)]
        dummy = cpool.tile([1, 8], F32, tag="dummy")

        # --- software-pipelined block loop: the basis front-end of block k
        # (sel matmuls, squares, cubes) is emitted BEFORE the G/out1 back-end
        # of block k-1, so the in-order PE stream never parks behind
        # dependent G matmuls while independent sel matmuls exist.
        def front(blk):
            Lb = blk // LBLK
            p0 = 32 * Lb
            csl = slice((blk % LBLK) * NB, (blk % LBLK + 1) * NB)
            xsm_b = xsm[p0:p0 + 16, csl]
            st = {"xsm_b": xsm_b, "p0": p0, "c2": [], "t_sb": []}
            # pairs of basis tiles in [128, 1024] two-bank PSUM supertiles:
            # (0,1) (2,3) (4,5) (6,-); one ScalarE Square per pair; cube STT
            # on DVE (pairs 0,3) or Pool via a PSUM->SBUF DMA bounce (1,2)
            for pi in range(4):
                tA, tB = 2 * pi, 2 * pi + 1
                nhalf = 1 if tB >= NTILE else 2
                w = nhalf * NB
                ps = pp_pair.tile([128, 2 * NB], F32, tag="pair")
                nc.tensor.matmul(ps[:, 0:NB], sel3[p0:p0 + 16, tA * 128:(tA + 1) * 128],
                                 xsm_b, start=True, stop=True)
                if nhalf == 2:
                    nc.tensor.matmul(ps[:, NB:2 * NB],
                                     sel3[p0:p0 + 16, tB * 128:(tB + 1) * 128],
                                     xsm_b, start=True, stop=True)
                c2 = spool.tile([128, 2 * NB], F32R, tag="c2")
                t_sb = spool.tile([128, 2 * NB], F32R, tag="t_sb")
                if pi == 2:
                    # Pool path (Pool cannot read PSUM): ScalarE bounces ell
                    # to SBUF, Pool squares and cubes it there
                    c_sb = dpool.tile([128, 2 * NB], F32, tag="c_sb")
                    nc.scalar.copy(c_sb[:, 0:w], ps[:, 0:w])
                    nc.gpsimd.tensor_mul(c2[:, 0:w], c_sb[:, 0:w], c_sb[:, 0:w])
                    nc.gpsimd.tensor_mul(t_sb[:, 0:w], c2[:, 0:w], c_sb[:, 0:w])
                else:
                    nc.scalar.activation(c2[:, 0:w], ps[:, 0:w], SQUARE)
                    # absorber takes the PE wait so the STT below only waits
                    # on ScalarE
                    nc.vector.tensor_copy(dummy[0:1, 0:1], ps[0:1, 0:1])
                    nc.vector.scalar_tensor_tensor(
                        t_sb[:, 0:w], ps[:, 0:w], 1.0, c2[:, 0:w], MULT, MULT)
                st["c2"].append(c2)
                st["t_sb"].append(t_sb)
            return st

        def back(blk, st):
            g_ps = pp_g.tile([64, NB], F32, tag="g")
            p0 = st["p0"]
            nc.tensor.matmul(g_ps[:], ux3[p0:p0 + 16], st["xsm_b"],
                             start=True, stop=False)
            for pi in range(4):
                tA, tB = 2 * pi, 2 * pi + 1
                nhalf = 1 if tB >= NTILE else 2
                c2, t_sb = st["c2"][pi], st["t_sb"][pi]
                for h, ti in enumerate((tA, tB)[:nhalf]):
                    nc.tensor.matmul(g_ps[:], ua[:, ti * 64:(ti + 1) * 64],
                                     t_sb[:, h * NB:(h + 1) * NB],
                                     start=False, stop=False)
                    if ti >= 5:   # quad fold reads c2 of tiles 5/6
                        nc.tensor.matmul(g_ps[:], ua[:, (2 + ti) * 64:(3 + ti) * 64],
                                         c2[:, h * NB:(h + 1) * NB],
                                         start=False, stop=ti == 6)

            # ---- transpose G per node, mix with Wrep', reduce kappa ----
            g_sb = fpool.tile([64, NB], F32, tag="g_sb")
            if blk % 2 == 0:
                nc.scalar.copy(g_sb[:], g_ps[:])
            else:
                nc.vector.tensor_copy(g_sb[:], g_ps[:])
            gt_ps = pp_misc.tile([C, NNOD * 64], F32, tag="misc")
            for bb in range(NNOD):
                nc.tensor.transpose(gt_ps[:, bb * 64:(bb + 1) * 64],
                                    g_sb[:, bb * C:(bb + 1) * C], ident[:64, :64])
            b0 = blk * NNOD
            p_sb = fpool.tile([C, NNOD * 64], F32, tag="p_sb")
            wr_v = wrep[:].rearrange("c (k b) -> c b k", k=64)[:, b0:b0 + NNOD, :]
            nc.vector.tensor_mul(p_sb[:].rearrange("c (b k) -> c b k", b=NNOD),
                                 gt_ps[:].rearrange("c (b k) -> c b k", b=NNOD), wr_v)
            nc.vector.tensor_reduce(
                out1[:, b0 * 4:(b0 + NNOD) * 4].rearrange("c (b l) -> c b l", l=4),
                p_sb[:].rearrange("c (b l k) -> c b l k", l=4, k=16),
                axis=AX.X, op=mybir.AluOpType.add)

        prev = None
        for blk in range(NBLK):
            st = front(blk)
            if prev is not None:
                back(*prev)
            prev = (blk, st)
        back(*prev)

        # ---- lin + tail ----
        _tail(nc, tc, fpool, pp_misc, out1, l0, l1, sct, ident, OUT, F32)
    nc.compile()
    return nc


def _tail(nc, tc, fpool, pp_misc, out1, l0, l1, sct, ident, OUT, F32):
        import concourse.mybir as mybir
        o1v = out1[:].rearrange("c (b l) -> c b l", l=4)
        lo_ps = pp_misc.tile([C, BLOC], F32, tag="misc")
        nc.tensor.matmul(lo_ps[:], l0[:], o1v[:, :, 0], start=True, stop=True)
        l1_ps = pp_misc.tile([C, BLOC * 3], F32, tag="misc")
        nc.tensor.matmul(l1_ps[:].rearrange("f (b d) -> f b d", d=3), l1[:],
                         o1v[:, :, 1:4], start=True, stop=True)
        lo_sb = fpool.tile([C, BLOC], F32, tag="lo_sb")
        nc.vector.tensor_copy(lo_sb[:], lo_ps[:])
        l1_sb = fpool.tile([C, BLOC * 3], F32, tag="l1_sb")
        nc.vector.tensor_copy(l1_sb[:], l1_ps[:])
        outt = fpool.tile([BLOC, 512], F32, tag="outt")
        tps = pp_misc.tile([BLOC, C], F32, tag="misc")
        nc.tensor.transpose(tps[:], lo_sb[:], ident[:])
        nc.vector.tensor_add(outt[:, 0:128], tps[:], sct[:, 0:128])
        l1v = l1_sb[:].rearrange("f (b d) -> f b d", d=3)
        o_v = outt[:, 128:].rearrange("b (f d) -> b f d", d=3)
        s_v = sct[:, 128:].rearrange("b (f d) -> b f d", d=3)
        for ddi in range(3):
            tpd = pp_misc.tile([BLOC, C], F32, tag="misc")
            nc.tensor.transpose(tpd[:], l1v[:, :, ddi], ident[:])
            nc.vector.tensor_add(o_v[:, :, ddi], tpd[:], s_v[:, :, ddi])
        nc.sync.dma_start(OUT[:], outt[:])


_PROG = {}


def kernel(**inputs):
    import concourse.bass_utils as bass_utils

    consts = _build_consts(inputs)

    nf = np.asarray(inputs["node_feats"], np.float32)
    attrs = np.asarray(inputs["node_attrs"], np.float32)
    sc = np.asarray(inputs["sc"], np.float32)

    if "prog" not in _PROG:
        _PROG["prog"] = build_program()
    nc = _PROG["prog"]

    in_maps = []
    for r in range(NCORES):
        b0 = r * BLOC
        xt = nf[b0:b0 + BLOC].transpose(2, 0, 1).reshape(16, NLOC)
        # 3-lane pack: lane Lb at partition base 32*Lb holds column blocks
        # [Lb*LBLK, (Lb+1)*LBLK)
        x3 = np.zeros((80, LANEW), np.float32)
        for blk in range(NBLK):
            Lb, cb = blk // LBLK, blk % LBLK
            x3[32 * Lb:32 * Lb + 16, cb * NB:(cb + 1) * NB] = xt[:, blk * NB:(blk + 1) * NB]
        m = {"X_Tm": x3,
             "attrsT": np.ascontiguousarray(attrs[b0:b0 + BLOC].T),
             "sc": np.ascontiguousarray(sc[b0:b0 + BLOC])}
        m.update(consts)
        in_maps.append(m)

    res = bass_utils.run_bass_kernel_spmd(
        nc, in_maps, list(range(NCORES)),
        trace=os.environ.get("KTRACE", "0") == "1")
    global LAST_EXEC_NS
    LAST_EXEC_NS = getattr(res, "exec_time_ns", None)
    outs = [np.asarray(res.results[r]["OUT"]) for r in range(NCORES)]
    return np.concatenate(outs, axis=0).astype(np.float32)


LAST_EXEC_NS = None


# revision 28
# speedup vs baseline: 1.2422x; 1.2422x over previous
"""Trainium2 Bass kernel for nn_EquivariantProductBasisBlock (MACE product basis).

Per (node b, channel c) the block computes a symmetric cubic polynomial in
x = node_feats[b,c,:] (16-dim). v2 basis: powers of linear forms.

  quad monomials q_ij = x_i x_j   -> spanned by squares   (a2*(x_i+x_j))^2
  cubic monomials t_ijm           -> spanned by cubes     (a3*(x_i+x_j+x_m))^3

The change of basis (A2/A3, cond ~6/21) is folded into U on the host in fp64,
so on-chip each basis tile is: one PE selection matmul (ell = Sel @ x), one
ScalarE Square (PSUM->SBUF), and for cubics one DVE/Pool scalar_tensor_tensor
(ell * ell^2). Basis tiles are paired into [128,1024] two-bank PSUM supertiles
so one ScalarE Square covers two tiles.

  G[(ld,kap), n] = Ufold.T @ [x; squares; cubes]      -- 10 PE matmuls / block
  Wrep[c,(kap,b)] = WK.T @ attrs.T (bf16 PE)          -- exact for dense attrs
  out1[c,(b,ld)]  = sum_kap G'[c,(b,ld,kap)] * Wrep'  -- transpose + VE
  out[b] = concat_li(lin_li.T @ out1)/sqrt(C) + sc

Sharding: data-parallel over nodes, 128 nodes/core on 8 cores, no collectives.
"""
import math
import os
import numpy as np

N, C, L, E = 1024, 128, 16, 10
NCORES = 8
BLOC = N // NCORES            # nodes per core
NLOC = BLOC * C               # (b,c) columns per core; n = b*C + c
NB = 512                      # column block (one fp32 PSUM bank)
NBLK = NLOC // NB
NNOD = NB // C                # nodes per block
LBLK = (NBLK + 2) // 3        # column blocks per partition lane (X packing)
LANEW = LBLK * NB             # free width per lane

PAIRS = [(i, j) for j in range(L) for i in range(j + 1)]              # 136
TRIPLES = [(i, j, m) for j in range(L) for i in range(j + 1) for m in range(j, L)]
NQ, NT = len(PAIRS), len(TRIPLES)                                      # 136, 816

# 7 cubic basis tiles of <=128 rows (base partition 0, zero-padded).  The 136
# "special" triples (i,j,15) live in tiles 5/6; their Act-Square intermediates
# (ell^2) span all quadratic monomials, so the quad U-path folds into two
# extra G matmuls reading c2 of tiles 5/6 — no dedicated quadratic tiles.
NTILE = 7
NSLOT = 9                     # U_all slots: 7 cube + 2 quad (c2 of tiles 5,6)

A2S = 1.0 / math.sqrt(2.0)    # scale for quad linear forms
A3S = 1.0 / math.sqrt(3.0)    # scale for cubic linear forms


def _build_consts(inputs):
    import itertools
    f32 = np.float32
    Us = [{nu: np.asarray(inputs[f"U_{li}_{nu}"], np.float64) for nu in (1, 2, 3)}
          for li in range(2)]
    lins = [np.asarray(inputs[f"lin_{li}"], f32) for li in range(2)]

    row_of_pair = {p: r for r, p in enumerate(PAIRS)}
    row_of_triple = {}
    for r, (i, j, m) in enumerate(TRIPLES):
        row_of_triple[tuple(sorted((i, j, m)))] = r

    # base U coefficients on monomial bases (as in the reference contraction)
    UX = np.zeros((16, 64), np.float64)
    Uq = np.zeros((NQ, 64), np.float64)
    U3 = np.zeros((NT, 64), np.float64)
    for ld in range(4):
        li, dd = (0, 0) if ld == 0 else (1, ld - 1)
        U3t, U2t, U1t = Us[li][3], Us[li][2], Us[li][1]
        UX[:, ld * 16 + 15] = U1t[dd, :, 0]
        for r, (i, j) in enumerate(PAIRS):
            v = U2t[dd, i, j, :] + (U2t[dd, j, i, :] if i != j else 0.0)
            Uq[r, ld * 16 + 11:ld * 16 + 15] = v
        for r, (i, j, m) in enumerate(TRIPLES):
            if i < j < m:
                arr = [(i, j, m), (i, m, j), (j, i, m), (j, m, i), (m, i, j), (m, j, i)]
            elif i == j and j < m:
                arr = [(i, i, m), (i, m, i), (m, i, i)]
            elif i < j and j == m:
                arr = [(i, j, j), (j, i, j), (j, j, i)]
            else:
                arr = [(i, i, i)]
            U3[r, ld * 16:ld * 16 + 11] = sum(U3t[dd, a, b, c, :] for (a, b, c) in arr)

    # cubic change of basis: y3 = A3 t  (y3_r = (a3(x_i+x_j+x_m))^3)
    A3 = np.zeros((NT, NT))
    for r, (i, j, m) in enumerate(TRIPLES):
        for (u, v, w) in itertools.product((i, j, m), repeat=3):
            A3[r, row_of_triple[tuple(sorted((u, v, w)))]] += 1.0
    U3f = np.linalg.solve(A3.T * (A3S ** 3), U3)

    # quad fold: q monomials via (a3(x_i+x_j+x_15))^2 of the special triples
    B = np.zeros((NQ, NQ))
    for r, (i, j) in enumerate(PAIRS):
        cv = np.zeros(16)
        cv[i] += A3S; cv[j] += A3S; cv[15] += A3S
        for a in range(16):
            for b in range(a, 16):
                coef = cv[a] * cv[b] * (2.0 if a != b else 1.0)
                if coef:
                    B[r, row_of_pair[(a, b)]] += coef
    Vq = np.linalg.solve(B.T, Uq)                    # [136, 64] on special forms

    # triple ordering: OTHER (m != 15) first, then SPECIAL = (i,j,15) in PAIRS
    # order.  Tiles: 0..4 = other[128k:128(k+1)], 5 = other[640:680]+special[0:88],
    # 6 = special[88:136] (zero-padded)
    special_orig = [row_of_triple[tuple(sorted((i, j, 15)))] for (i, j) in PAIRS]
    other_orig = [r for r, t in enumerate(TRIPLES) if t[2] != 15]
    assert len(other_orig) == NT - NQ

    def tile_row(ti, p):
        """('o'|'s', idx) of the basis row at partition p of tile ti, or None"""
        if ti < 5:
            return ("o", ti * 128 + p)
        if ti == 5:
            return ("o", 640 + p) if p < 40 else ("s", p - 40)
        return ("s", 88 + p) if p < 48 else None

    SEL = np.zeros((16, NTILE * 128), np.float64)
    U_all = np.zeros((128, 64 * NSLOT), np.float64)
    for ti in range(NTILE):
        for p in range(128):
            r = tile_row(ti, p)
            if r is None:
                continue
            kind, k = r
            orig = other_orig[k] if kind == "o" else special_orig[k]
            i, j, m = TRIPLES[orig]
            SEL[i, ti * 128 + p] += A3S
            SEL[j, ti * 128 + p] += A3S
            SEL[m, ti * 128 + p] += A3S
            U_all[p, ti * 64:(ti + 1) * 64] = U3f[orig]
            if kind == "s":
                U_all[p, (7 + (ti - 5)) * 64:(8 + (ti - 5)) * 64] = Vq[k]

    # 3-lane packing at partition bases {0,32,64} (lhsT.base == rhs.base)
    def lane3(mat):
        rows = mat.shape[0]
        out = np.zeros((64 + rows, mat.shape[1]), mat.dtype)
        for Lb in range(3):
            out[32 * Lb:32 * Lb + rows] = mat
        return out

    # WK packed at 3 bases: kappa groups 0..23 | 24..47 | 48..63
    Ws = [{nu: np.asarray(inputs[f"W_{li}_{nu}"], f32) for nu in (1, 2, 3)}
          for li in range(2)]
    WKp = np.zeros((E, 64, C), f32)
    for ld in range(4):
        li = 0 if ld == 0 else 1
        WKp[:, ld * 16:ld * 16 + 11, :] = Ws[li][3]
        WKp[:, ld * 16 + 11:ld * 16 + 15, :] = Ws[li][2]
        WKp[:, ld * 16 + 15, :] = Ws[li][1][:, 0, :]
    WK3 = np.zeros((74, 24 * C), f32)
    for kap in range(64):
        g, off = (0, 0) if kap < 24 else ((1, 24) if kap < 48 else (2, 48))
        WK3[32 * g:32 * g + E, (kap - off) * C:(kap - off + 1) * C] = WKp[:, kap, :]

    isc = f32(1.0 / math.sqrt(C))
    return {
        "U_all": U_all.astype(f32),
        "UX3": lane3(UX.astype(f32)),
        "SEL3": lane3(SEL.astype(f32)),
        "WK3": WK3,
        "lin0": np.ascontiguousarray(lins[0] * isc),
        "lin1": np.ascontiguousarray(lins[1] * isc),
    }


def build_program():
    import concourse.bass as bass
    import concourse.bacc as bacc
    import concourse.mybir as mybir
    import concourse.tile as tile
    from concourse.masks import make_identity
    from contextlib import ExitStack

    dt = mybir.dt
    F32 = dt.float32
    F32R = dt.float32r
    BF16 = dt.bfloat16
    AX = mybir.AxisListType
    SQUARE = mybir.ActivationFunctionType.Square
    MULT = mybir.AluOpType.mult

    nc = bacc.Bacc(None, target_bir_lowering=False)
    X_Tm = nc.dram_tensor("X_Tm", [80, LANEW], F32, kind="ExternalInput")
    attrsT = nc.dram_tensor("attrsT", [E, BLOC], F32, kind="ExternalInput")
    sc_d = nc.dram_tensor("sc", [BLOC, 512], F32, kind="ExternalInput")
    U_all = nc.dram_tensor("U_all", [128, 64 * NSLOT], F32, kind="ExternalInput")
    UX3 = nc.dram_tensor("UX3", [80, 64], F32, kind="ExternalInput")
    SEL3 = nc.dram_tensor("SEL3", [80, NTILE * 128], F32, kind="ExternalInput")
    WK3 = nc.dram_tensor("WK3", [74, 24 * C], F32, kind="ExternalInput")
    lin0 = nc.dram_tensor("lin0", [C, C], F32, kind="ExternalInput")
    lin1 = nc.dram_tensor("lin1", [C, C], F32, kind="ExternalInput")
    OUT = nc.dram_tensor("OUT", [BLOC, 512], F32, kind="ExternalOutput")

    with tile.TileContext(nc) as tc, ExitStack() as ctx:
        cpool = ctx.enter_context(tc.tile_pool(name="consts", bufs=1))
        fpool = ctx.enter_context(tc.tile_pool(name="feats", bufs=2))
        spool = ctx.enter_context(tc.tile_pool(name="stream", bufs=8))
        dpool = ctx.enter_context(tc.tile_pool(name="dmab", bufs=4))
        # PSUM (8 banks): pair supertile (2 banks x 3 bufs) + g (1x1) + misc (1x1)
        pp_pair = ctx.enter_context(tc.tile_pool(name="ps_pair", bufs=3, space="PSUM"))
        pp_g = ctx.enter_context(tc.tile_pool(name="ps_g", bufs=1, space="PSUM"))
        pp_misc = ctx.enter_context(tc.tile_pool(name="ps_misc", bufs=1, space="PSUM"))

        # PE-consumed tiles laundered through one copy each so matmul operand
        # producers collapse onto a single engine.
        def launder(shape, dtp, tag, src):
            raw = cpool.tile(shape, src.dtype, tag=tag + "_r")
            nc.sync.dma_start(raw[:], src[:])
            t = cpool.tile(shape, dtp, tag=tag)
            nc.vector.tensor_copy(t[:], raw[:])
            return t

        xsm = launder([80, LANEW], F32R, "xTm", X_Tm)
        ua = launder([128, 64 * NSLOT], F32R, "uall", U_all)
        ux3 = launder([80, 64], F32R, "ux3", UX3)
        sel3 = launder([80, NTILE * 128], F32R, "sel3", SEL3)
        wk3 = launder([74, 24 * C], BF16, "wk3", WK3)
        l0 = launder([C, C], F32, "lin0", lin0)
        l1 = launder([C, C], F32, "lin1", lin1)
        # attrs replicated at the 3 bases to pair with WK3 lhsT slices
        ats_raw = cpool.tile([74, BLOC], F32, tag="attrs_r")
        ats = cpool.tile([74, BLOC], BF16, tag="attrs")
        for Lb in range(3):
            nc.sync.dma_start(ats_raw[32 * Lb:32 * Lb + E], attrsT[:])
            nc.vector.tensor_copy(ats[32 * Lb:32 * Lb + E], ats_raw[32 * Lb:32 * Lb + E])
        sct = cpool.tile([BLOC, 512], F32, tag="sc"); nc.sync.dma_start(sct[:], sc_d[:])
        ident_raw = cpool.tile([128, 128], F32, tag="ident_r")
        make_identity(nc, ident_raw[:])
        ident = cpool.tile([128, 128], F32, tag="ident")
        nc.vector.tensor_copy(ident[:], ident_raw[:])

        # Wrep' [c, (kap, b)]: 16 rounds x 4 kappa, K=10 bf16 matmuls
        wrep = cpool.tile([C, 64 * BLOC], F32, tag="wrep")
        for rnd in range(16):
            wps = pp_misc.tile([C, 4 * BLOC], F32, tag="misc")
            for kk in range(4):
                kap = rnd * 4 + kk
                g3, off = (0, 0) if kap < 24 else ((1, 24) if kap < 48 else (2, 48))
                nc.tensor.matmul(
                    wps[:, kk * BLOC:(kk + 1) * BLOC],
                    wk3[32 * g3:32 * g3 + E, (kap - off) * C:(kap - off + 1) * C],
                    ats[32 * g3:32 * g3 + E], start=True, stop=True)
            nc.scalar.copy(wrep[:, rnd * 4 * BLOC:(rnd + 1) * 4 * BLOC], wps[:])

        out1 = cpool.tile([C, BLOC * 4], F32, tag="out1")  # [c, (b, ld)]
        dummy = cpool.tile([1, 8], F32, tag="dummy")

        # --- software-pipelined block loop: the basis front-end of block k
        # (sel matmuls, squares, cubes) is emitted BEFORE the G/out1 back-end
        # of block k-1, so the in-order PE stream never parks behind
        # dependent G matmuls while independent sel matmuls exist.
        def front(blk):
            Lb = blk // LBLK
            p0 = 32 * Lb
            csl = slice((blk % LBLK) * NB, (blk % LBLK + 1) * NB)
            xsm_b = xsm[p0:p0 + 16, csl]
            st = {"xsm_b": xsm_b, "p0": p0, "c2": [], "t_sb": []}
            # pairs of basis tiles in [128, 1024] two-bank PSUM supertiles:
            # (0,1) (2,3) (4,5) (6,-); one ScalarE Square per pair; cube STT
            # on DVE (pairs 0,3) or Pool via a PSUM->SBUF DMA bounce (1,2)
            for pi in range(4):
                tA, tB = 2 * pi, 2 * pi + 1
                nhalf = 1 if tB >= NTILE else 2
                w = nhalf * NB
                ps = pp_pair.tile([128, 2 * NB], F32, tag="pair")
                nc.tensor.matmul(ps[:, 0:NB], sel3[p0:p0 + 16, tA * 128:(tA + 1) * 128],
                                 xsm_b, start=True, stop=True)
                if nhalf == 2:
                    nc.tensor.matmul(ps[:, NB:2 * NB],
                                     sel3[p0:p0 + 16, tB * 128:(tB + 1) * 128],
                                     xsm_b, start=True, stop=True)
                c2 = spool.tile([128, 2 * NB], F32R, tag="c2")
                t_sb = spool.tile([128, 2 * NB], F32R, tag="t_sb")
                if pi == 2:
                    # Pool path (Pool cannot read PSUM): ScalarE bounces ell
                    # to SBUF, Pool squares and cubes it there
                    c_sb = dpool.tile([128, 2 * NB], F32, tag="c_sb")
                    nc.scalar.copy(c_sb[:, 0:w], ps[:, 0:w])
                    nc.gpsimd.tensor_mul(c2[:, 0:w], c_sb[:, 0:w], c_sb[:, 0:w])
                    nc.gpsimd.tensor_mul(t_sb[:, 0:w], c2[:, 0:w], c_sb[:, 0:w])
                else:
                    nc.scalar.activation(c2[:, 0:w], ps[:, 0:w], SQUARE)
                    # absorber takes the PE wait so the STT below only waits
                    # on ScalarE
                    nc.vector.tensor_copy(dummy[0:1, 0:1], ps[0:1, 0:1])
                    nc.vector.scalar_tensor_tensor(
                        t_sb[:, 0:w], ps[:, 0:w], 1.0, c2[:, 0:w], MULT, MULT)
                st["c2"].append(c2)
                st["t_sb"].append(t_sb)
            return st

        def back(blk, st):
            g_ps = pp_g.tile([64, NB], F32, tag="g")
            p0 = st["p0"]
            nc.tensor.matmul(g_ps[:], ux3[p0:p0 + 16], st["xsm_b"],
                             start=True, stop=False)
            for pi in range(4):
                tA, tB = 2 * pi, 2 * pi + 1
                nhalf = 1 if tB >= NTILE else 2
                c2, t_sb = st["c2"][pi], st["t_sb"][pi]
                for h, ti in enumerate((tA, tB)[:nhalf]):
                    nc.tensor.matmul(g_ps[:], ua[:, ti * 64:(ti + 1) * 64],
                                     t_sb[:, h * NB:(h + 1) * NB],
                                     start=False, stop=False)
                    if ti >= 5:   # quad fold reads c2 of tiles 5/6
                        nc.tensor.matmul(g_ps[:], ua[:, (2 + ti) * 64:(3 + ti) * 64],
                                         c2[:, h * NB:(h + 1) * NB],
                                         start=False, stop=ti == 6)

            # ---- transpose G per node, mix with Wrep', reduce kappa ----
            g_sb = fpool.tile([64, NB], F32, tag="g_sb")
            if blk % 2 == 0:
                nc.scalar.copy(g_sb[:], g_ps[:])
            else:
                nc.vector.tensor_copy(g_sb[:], g_ps[:])
            gt_ps = pp_misc.tile([C, NNOD * 64], F32, tag="misc")
            for bb in range(NNOD):
                nc.tensor.transpose(gt_ps[:, bb * 64:(bb + 1) * 64],
                                    g_sb[:, bb * C:(bb + 1) * C], ident[:64, :64])
            b0 = blk * NNOD
            p_sb = fpool.tile([C, NNOD * 64], F32, tag="p_sb")
            wr_v = wrep[:].rearrange("c (k b) -> c b k", k=64)[:, b0:b0 + NNOD, :]
            nc.vector.tensor_mul(p_sb[:].rearrange("c (b k) -> c b k", b=NNOD),
                                 gt_ps[:].rearrange("c (b k) -> c b k", b=NNOD), wr_v)
            nc.vector.tensor_reduce(
                out1[:, b0 * 4:(b0 + NNOD) * 4].rearrange("c (b l) -> c b l", l=4),
                p_sb[:].rearrange("c (b l k) -> c b l k", l=4, k=16),
                axis=AX.X, op=mybir.AluOpType.add)

        prev = None
        for blk in range(NBLK):
            st = front(blk)
            if prev is not None:
                back(*prev)
            prev = (blk, st)
        back(*prev)

        # ---- lin + tail ----
        _tail(nc, tc, fpool, pp_misc, out1, l0, l1, sct, ident, OUT, F32)
    nc.compile()
    return nc


def _tail(nc, tc, fpool, pp_misc, out1, l0, l1, sct, ident, OUT, F32):
        import concourse.mybir as mybir
        o1v = out1[:].rearrange("c (b l) -> c b l", l=4)
        lo_ps = pp_misc.tile([C, BLOC], F32, tag="misc")
        nc.tensor.matmul(lo_ps[:], l0[:], o1v[:, :, 0], start=True, stop=True)
        l1_ps = pp_misc.tile([C, BLOC * 3], F32, tag="misc")
        nc.tensor.matmul(l1_ps[:].rearrange("f (b d) -> f b d", d=3), l1[:],
                         o1v[:, :, 1:4], start=True, stop=True)
        lo_sb = fpool.tile([C, BLOC], F32, tag="lo_sb")
        nc.vector.tensor_copy(lo_sb[:], lo_ps[:])
        l1_sb = fpool.tile([C, BLOC * 3], F32, tag="l1_sb")
        nc.vector.tensor_copy(l1_sb[:], l1_ps[:])
        outt = fpool.tile([BLOC, 512], F32, tag="outt")
        tps = pp_misc.tile([BLOC, C], F32, tag="misc")
        nc.tensor.transpose(tps[:], lo_sb[:], ident[:])
        nc.vector.tensor_add(outt[:, 0:128], tps[:], sct[:, 0:128])
        l1v = l1_sb[:].rearrange("f (b d) -> f b d", d=3)
        o_v = outt[:, 128:].rearrange("b (f d) -> b f d", d=3)
        s_v = sct[:, 128:].rearrange("b (f d) -> b f d", d=3)
        for ddi in range(3):
            tpd = pp_misc.tile([BLOC, C], F32, tag="misc")
            nc.tensor.transpose(tpd[:], l1v[:, :, ddi], ident[:])
            nc.vector.tensor_add(o_v[:, :, ddi], tpd[:], s_v[:, :, ddi])
        nc.sync.dma_start(OUT[:], outt[:])


_PROG = {}


def kernel(**inputs):
    import concourse.bass_utils as bass_utils

    consts = _build_consts(inputs)

    nf = np.asarray(inputs["node_feats"], np.float32)
    attrs = np.asarray(inputs["node_attrs"], np.float32)
    sc = np.asarray(inputs["sc"], np.float32)

    if "prog" not in _PROG:
        _PROG["prog"] = build_program()
    nc = _PROG["prog"]

    in_maps = []
    for r in range(NCORES):
        b0 = r * BLOC
        xt = nf[b0:b0 + BLOC].transpose(2, 0, 1).reshape(16, NLOC)
        # 3-lane pack: lane Lb at partition base 32*Lb holds column blocks
        # [Lb*LBLK, (Lb+1)*LBLK)
        x3 = np.zeros((80, LANEW), np.float32)
        for blk in range(NBLK):
            Lb, cb = blk // LBLK, blk % LBLK
            x3[32 * Lb:32 * Lb + 16, cb * NB:(cb + 1) * NB] = xt[:, blk * NB:(blk + 1) * NB]
        m = {"X_Tm": x3,
             "attrsT": np.ascontiguousarray(attrs[b0:b0 + BLOC].T),
             "sc": np.ascontiguousarray(sc[b0:b0 + BLOC])}
        m.update(consts)
        in_maps.append(m)

    res = bass_utils.run_bass_kernel_spmd(
        nc, in_maps, list(range(NCORES)),
        trace=os.environ.get("KTRACE", "0") == "1")
    global LAST_EXEC_NS
    LAST_EXEC_NS = getattr(res, "exec_time_ns", None)
    outs = [np.asarray(res.results[r]["OUT"]) for r in range(NCORES)]
    return np.concatenate(outs, axis=0).astype(np.float32)


LAST_EXEC_NS = None


# revision 51
# speedup vs baseline: 1.2614x; 1.0154x over previous
"""Trainium2 Bass kernel for nn_EquivariantProductBasisBlock (MACE product basis).

Per (node b, channel c) the block computes a symmetric cubic polynomial in
x = node_feats[b,c,:] (16-dim). v2 basis: powers of linear forms.

  quad monomials q_ij = x_i x_j   -> spanned by squares   (a2*(x_i+x_j))^2
  cubic monomials t_ijm           -> spanned by cubes     (a3*(x_i+x_j+x_m))^3

The change of basis (A2/A3, cond ~6/21) is folded into U on the host in fp64,
so on-chip each basis tile is: one PE selection matmul (ell = Sel @ x), one
ScalarE Square (PSUM->SBUF), and for cubics one DVE/Pool scalar_tensor_tensor
(ell * ell^2). Basis tiles are paired into [128,1024] two-bank PSUM supertiles
so one ScalarE Square covers two tiles.

  G[(ld,kap), n] = Ufold.T @ [x; squares; cubes]      -- 10 PE matmuls / block
  Wrep[c,(kap,b)] = WK.T @ attrs.T (bf16 PE)          -- exact for dense attrs
  out1[c,(b,ld)]  = sum_kap G'[c,(b,ld,kap)] * Wrep'  -- transpose + VE
  out[b] = concat_li(lin_li.T @ out1)/sqrt(C) + sc

Sharding: data-parallel over nodes, 128 nodes/core on 8 cores, no collectives.
"""
import math
import os
import numpy as np

N, C, L, E = 1024, 128, 16, 10
NCORES = 8
BLOC = N // NCORES            # nodes per core
NLOC = BLOC * C               # (b,c) columns per core; n = b*C + c
NB = 512                      # column block (one fp32 PSUM bank)
NBLK = NLOC // NB
NNOD = NB // C                # nodes per block
LBLK = (NBLK + 2) // 3        # column blocks per partition lane (X packing)
LANEW = LBLK * NB             # free width per lane

PAIRS = [(i, j) for j in range(L) for i in range(j + 1)]              # 136
TRIPLES = [(i, j, m) for j in range(L) for i in range(j + 1) for m in range(j, L)]
NQ, NT = len(PAIRS), len(TRIPLES)                                      # 136, 816

# 7 cubic basis tiles of <=128 rows (base partition 0, zero-padded).  The 136
# "special" triples (i,j,15) live in tiles 5/6; their Act-Square intermediates
# (ell^2) span all quadratic monomials, so the quad U-path folds into two
# extra G matmuls reading c2 of tiles 5/6 — no dedicated quadratic tiles.
NTILE = 7
NSLOT = 9                     # U_all slots: 7 cube + 2 quad (c2 of tiles 5,6)

A2S = 1.0 / math.sqrt(2.0)    # scale for quad linear forms
A3S = 1.0 / math.sqrt(3.0)    # scale for cubic linear forms


def _build_consts(inputs):
    import itertools
    f32 = np.float32
    Us = [{nu: np.asarray(inputs[f"U_{li}_{nu}"], np.float64) for nu in (1, 2, 3)}
          for li in range(2)]
    lins = [np.asarray(inputs[f"lin_{li}"], f32) for li in range(2)]

    row_of_pair = {p: r for r, p in enumerate(PAIRS)}
    row_of_triple = {}
    for r, (i, j, m) in enumerate(TRIPLES):
        row_of_triple[tuple(sorted((i, j, m)))] = r

    # base U coefficients on monomial bases (as in the reference contraction)
    UX = np.zeros((16, 64), np.float64)
    Uq = np.zeros((NQ, 64), np.float64)
    U3 = np.zeros((NT, 64), np.float64)
    for ld in range(4):
        li, dd = (0, 0) if ld == 0 else (1, ld - 1)
        U3t, U2t, U1t = Us[li][3], Us[li][2], Us[li][1]
        UX[:, ld * 16 + 15] = U1t[dd, :, 0]
        for r, (i, j) in enumerate(PAIRS):
            v = U2t[dd, i, j, :] + (U2t[dd, j, i, :] if i != j else 0.0)
            Uq[r, ld * 16 + 11:ld * 16 + 15] = v
        for r, (i, j, m) in enumerate(TRIPLES):
            if i < j < m:
                arr = [(i, j, m), (i, m, j), (j, i, m), (j, m, i), (m, i, j), (m, j, i)]
            elif i == j and j < m:
                arr = [(i, i, m), (i, m, i), (m, i, i)]
            elif i < j and j == m:
                arr = [(i, j, j), (j, i, j), (j, j, i)]
            else:
                arr = [(i, i, i)]
            U3[r, ld * 16:ld * 16 + 11] = sum(U3t[dd, a, b, c, :] for (a, b, c) in arr)

    # cubic change of basis: y3 = A3 t  (y3_r = (a3(x_i+x_j+x_m))^3)
    A3 = np.zeros((NT, NT))
    for r, (i, j, m) in enumerate(TRIPLES):
        for (u, v, w) in itertools.product((i, j, m), repeat=3):
            A3[r, row_of_triple[tuple(sorted((u, v, w)))]] += 1.0
    U3f = np.linalg.solve(A3.T * (A3S ** 3), U3)

    # quad fold: q monomials via (a3(x_i+x_j+x_15))^2 of the special triples
    B = np.zeros((NQ, NQ))
    for r, (i, j) in enumerate(PAIRS):
        cv = np.zeros(16)
        cv[i] += A3S; cv[j] += A3S; cv[15] += A3S
        for a in range(16):
            for b in range(a, 16):
                coef = cv[a] * cv[b] * (2.0 if a != b else 1.0)
                if coef:
                    B[r, row_of_pair[(a, b)]] += coef
    Vq = np.linalg.solve(B.T, Uq)                    # [136, 64] on special forms

    # triple ordering: OTHER (m != 15) first, then SPECIAL = (i,j,15) in PAIRS
    # order.  Tiles: 0..4 = other[128k:128(k+1)], 5 = other[640:680]+special[0:88],
    # 6 = special[88:136] (zero-padded)
    special_orig = [row_of_triple[tuple(sorted((i, j, 15)))] for (i, j) in PAIRS]
    other_orig = [r for r, t in enumerate(TRIPLES) if t[2] != 15]
    assert len(other_orig) == NT - NQ

    def tile_row(ti, p):
        """('o'|'s', idx) of the basis row at partition p of tile ti, or None"""
        if ti < 5:
            return ("o", ti * 128 + p)
        if ti == 5:
            return ("o", 640 + p) if p < 40 else ("s", p - 40)
        return ("s", 88 + p) if p < 48 else None

    SEL = np.zeros((16, NTILE * 128), np.float64)
    U_all = np.zeros((128, 64 * NSLOT), np.float64)
    for ti in range(NTILE):
        for p in range(128):
            r = tile_row(ti, p)
            if r is None:
                continue
            kind, k = r
            orig = other_orig[k] if kind == "o" else special_orig[k]
            i, j, m = TRIPLES[orig]
            SEL[i, ti * 128 + p] += A3S
            SEL[j, ti * 128 + p] += A3S
            SEL[m, ti * 128 + p] += A3S
            U_all[p, ti * 64:(ti + 1) * 64] = U3f[orig]
            if kind == "s":
                U_all[p, (7 + (ti - 5)) * 64:(8 + (ti - 5)) * 64] = Vq[k]

    # 3-lane packing at partition bases {0,32,64} (lhsT.base == rhs.base)
    def lane3(mat):
        rows = mat.shape[0]
        out = np.zeros((64 + rows, mat.shape[1]), mat.dtype)
        for Lb in range(3):
            out[32 * Lb:32 * Lb + rows] = mat
        return out

    # WK packed at 3 bases: kappa groups 0..23 | 24..47 | 48..63
    Ws = [{nu: np.asarray(inputs[f"W_{li}_{nu}"], f32) for nu in (1, 2, 3)}
          for li in range(2)]
    WKp = np.zeros((E, 64, C), f32)
    for ld in range(4):
        li = 0 if ld == 0 else 1
        WKp[:, ld * 16:ld * 16 + 11, :] = Ws[li][3]
        WKp[:, ld * 16 + 11:ld * 16 + 15, :] = Ws[li][2]
        WKp[:, ld * 16 + 15, :] = Ws[li][1][:, 0, :]
    WK3 = np.zeros((74, 24 * C), f32)
    for kap in range(64):
        g, off = (0, 0) if kap < 24 else ((1, 24) if kap < 48 else (2, 48))
        WK3[32 * g:32 * g + E, (kap - off) * C:(kap - off + 1) * C] = WKp[:, kap, :]

    isc = f32(1.0 / math.sqrt(C))
    return {
        "U_all": U_all.astype(f32),
        "UX3": lane3(UX.astype(f32)),
        "SEL3": lane3(SEL.astype(f32)),
        "WK3": WK3,
        "lin0": np.ascontiguousarray(lins[0] * isc),
        "lin1": np.ascontiguousarray(lins[1] * isc),
    }


def build_program():
    import concourse.bass as bass
    import concourse.bacc as bacc
    import concourse.mybir as mybir
    import concourse.tile as tile
    from concourse.masks import make_identity
    from contextlib import ExitStack

    dt = mybir.dt
    F32 = dt.float32
    F32R = dt.float32r
    BF16 = dt.bfloat16
    AX = mybir.AxisListType
    SQUARE = mybir.ActivationFunctionType.Square
    MULT = mybir.AluOpType.mult

    nc = bacc.Bacc(None, target_bir_lowering=False)
    X_Tm = nc.dram_tensor("X_Tm", [80, LANEW], F32, kind="ExternalInput")
    attrsT = nc.dram_tensor("attrsT", [E, BLOC], F32, kind="ExternalInput")
    sc_d = nc.dram_tensor("sc", [BLOC, 512], F32, kind="ExternalInput")
    U_all = nc.dram_tensor("U_all", [128, 64 * NSLOT], F32, kind="ExternalInput")
    UX3 = nc.dram_tensor("UX3", [80, 64], F32, kind="ExternalInput")
    SEL3 = nc.dram_tensor("SEL3", [80, NTILE * 128], F32, kind="ExternalInput")
    WK3 = nc.dram_tensor("WK3", [74, 24 * C], F32, kind="ExternalInput")
    lin0 = nc.dram_tensor("lin0", [C, C], F32, kind="ExternalInput")
    lin1 = nc.dram_tensor("lin1", [C, C], F32, kind="ExternalInput")
    OUT = nc.dram_tensor("OUT", [BLOC, 512], F32, kind="ExternalOutput")

    with tile.TileContext(nc) as tc, ExitStack() as ctx:
        cpool = ctx.enter_context(tc.tile_pool(name="consts", bufs=1))
        fpool = ctx.enter_context(tc.tile_pool(name="feats", bufs=2))
        spool = ctx.enter_context(tc.tile_pool(name="stream", bufs=8))
        dpool = ctx.enter_context(tc.tile_pool(name="dmab", bufs=4))
        # PSUM (8 banks): pair supertile (2 banks x 3 bufs) + g (1x1) + misc (1x1)
        pp_pair = ctx.enter_context(tc.tile_pool(name="ps_pair", bufs=3, space="PSUM"))
        pp_g = ctx.enter_context(tc.tile_pool(name="ps_g", bufs=1, space="PSUM"))
        pp_misc = ctx.enter_context(tc.tile_pool(name="ps_misc", bufs=1, space="PSUM"))

        # PE-consumed tiles laundered through one copy each so matmul operand
        # producers collapse onto a single engine.
        def launder(shape, dtp, tag, src):
            raw = cpool.tile(shape, src.dtype, tag=tag + "_r")
            nc.sync.dma_start(raw[:], src[:])
            t = cpool.tile(shape, dtp, tag=tag)
            nc.vector.tensor_copy(t[:], raw[:])
            return t

        xsm = launder([80, LANEW], F32R, "xTm", X_Tm)
        ua = launder([128, 64 * NSLOT], F32R, "uall", U_all)
        ux3 = launder([80, 64], F32R, "ux3", UX3)
        sel3 = launder([80, NTILE * 128], F32R, "sel3", SEL3)
        wk3 = launder([74, 24 * C], BF16, "wk3", WK3)
        l0 = launder([C, C], F32, "lin0", lin0)
        l1 = launder([C, C], F32, "lin1", lin1)
        # attrs replicated at the 3 bases to pair with WK3 lhsT slices
        ats_raw = cpool.tile([74, BLOC], F32, tag="attrs_r")
        ats = cpool.tile([74, BLOC], BF16, tag="attrs")
        for Lb in range(3):
            nc.sync.dma_start(ats_raw[32 * Lb:32 * Lb + E], attrsT[:])
            nc.vector.tensor_copy(ats[32 * Lb:32 * Lb + E], ats_raw[32 * Lb:32 * Lb + E])
        sct = cpool.tile([BLOC, 512], F32, tag="sc"); nc.sync.dma_start(sct[:], sc_d[:])
        ident_raw = cpool.tile([128, 128], F32, tag="ident_r")
        make_identity(nc, ident_raw[:])
        ident = cpool.tile([128, 128], F32, tag="ident")
        nc.vector.tensor_copy(ident[:], ident_raw[:])

        # Wrep' [c, (kap, b)]: 16 rounds x 4 kappa, K=10 bf16 matmuls
        wrep = cpool.tile([C, 64 * BLOC], F32, tag="wrep")
        for rnd in range(16):
            wps = pp_misc.tile([C, 4 * BLOC], F32, tag="misc")
            for kk in range(4):
                kap = rnd * 4 + kk
                g3, off = (0, 0) if kap < 24 else ((1, 24) if kap < 48 else (2, 48))
                nc.tensor.matmul(
                    wps[:, kk * BLOC:(kk + 1) * BLOC],
                    wk3[32 * g3:32 * g3 + E, (kap - off) * C:(kap - off + 1) * C],
                    ats[32 * g3:32 * g3 + E], start=True, stop=True)
            nc.scalar.copy(wrep[:, rnd * 4 * BLOC:(rnd + 1) * 4 * BLOC], wps[:])

        out1 = cpool.tile([C, BLOC * 4], F32, tag="out1")  # [c, (b, ld)]
        dummy = cpool.tile([1, 8], F32, tag="dummy")

        # --- software-pipelined block loop: the basis front-end of block k
        # (sel matmuls, squares, cubes) is emitted BEFORE the G/out1 back-end
        # of block k-1, so the in-order PE stream never parks behind
        # dependent G matmuls while independent sel matmuls exist.
        def front(blk):
            Lb = blk // LBLK
            p0 = 32 * Lb
            csl = slice((blk % LBLK) * NB, (blk % LBLK + 1) * NB)
            xsm_b = xsm[p0:p0 + 16, csl]
            st = {"xsm_b": xsm_b, "p0": p0, "c2": [], "t_sb": []}
            # pairs of basis tiles in [128, 1024] two-bank PSUM supertiles:
            # (0,1) (2,3) (4,5) (6,-); one ScalarE Square per pair; cube STT
            # on DVE (pairs 0,3) or Pool via a PSUM->SBUF DMA bounce (1,2)
            for pi in range(4):
                tA, tB = 2 * pi, 2 * pi + 1
                nhalf = 1 if tB >= NTILE else 2
                w = nhalf * NB
                ps = pp_pair.tile([128, 2 * NB], F32, tag="pair")
                nc.tensor.matmul(ps[:, 0:NB], sel3[p0:p0 + 16, tA * 128:(tA + 1) * 128],
                                 xsm_b, start=True, stop=True)
                if nhalf == 2:
                    nc.tensor.matmul(ps[:, NB:2 * NB],
                                     sel3[p0:p0 + 16, tB * 128:(tB + 1) * 128],
                                     xsm_b, start=True, stop=True)
                c2 = spool.tile([128, 2 * NB], F32R, tag="c2")
                t_sb = spool.tile([128, 2 * NB], F32R, tag="t_sb")
                if pi == 2:
                    # Pool path (Pool cannot read PSUM): ScalarE bounces ell
                    # to SBUF, Pool squares and cubes it there
                    c_sb = dpool.tile([128, 2 * NB], F32, tag="c_sb")
                    nc.scalar.copy(c_sb[:, 0:w], ps[:, 0:w])
                    nc.gpsimd.tensor_mul(c2[:, 0:w], c_sb[:, 0:w], c_sb[:, 0:w])
                    nc.gpsimd.tensor_mul(t_sb[:, 0:w], c2[:, 0:w], c_sb[:, 0:w])
                else:
                    nc.scalar.activation(c2[:, 0:w], ps[:, 0:w], SQUARE)
                    nc.vector.scalar_tensor_tensor(
                        t_sb[:, 0:w], ps[:, 0:w], 1.0, c2[:, 0:w], MULT, MULT)
                st["c2"].append(c2)
                st["t_sb"].append(t_sb)
            return st

        def back(blk, st):
            g_ps = pp_g.tile([64, NB], F32, tag="g")
            p0 = st["p0"]
            nc.tensor.matmul(g_ps[:], ux3[p0:p0 + 16], st["xsm_b"],
                             start=True, stop=False)
            for pi in range(4):
                tA, tB = 2 * pi, 2 * pi + 1
                nhalf = 1 if tB >= NTILE else 2
                c2, t_sb = st["c2"][pi], st["t_sb"][pi]
                for h, ti in enumerate((tA, tB)[:nhalf]):
                    nc.tensor.matmul(g_ps[:], ua[:, ti * 64:(ti + 1) * 64],
                                     t_sb[:, h * NB:(h + 1) * NB],
                                     start=False, stop=False)
                    if ti >= 5:   # quad fold reads c2 of tiles 5/6
                        nc.tensor.matmul(g_ps[:], ua[:, (2 + ti) * 64:(3 + ti) * 64],
                                         c2[:, h * NB:(h + 1) * NB],
                                         start=False, stop=ti == 6)

            # ---- transpose G per node, mix with Wrep', reduce kappa ----
            g_sb = fpool.tile([64, NB], F32, tag="g_sb")
            if blk % 2 == 0:
                nc.scalar.copy(g_sb[:], g_ps[:])
            else:
                nc.vector.tensor_copy(g_sb[:], g_ps[:])
            gt_ps = pp_misc.tile([C, NNOD * 64], F32, tag="misc")
            for bb in range(NNOD):
                nc.tensor.transpose(gt_ps[:, bb * 64:(bb + 1) * 64],
                                    g_sb[:, bb * C:(bb + 1) * C], ident[:64, :64])
            b0 = blk * NNOD
            p_sb = fpool.tile([C, NNOD * 64], F32, tag="p_sb")
            wr_v = wrep[:].rearrange("c (k b) -> c b k", k=64)[:, b0:b0 + NNOD, :]
            nc.vector.tensor_mul(p_sb[:].rearrange("c (b k) -> c b k", b=NNOD),
                                 gt_ps[:].rearrange("c (b k) -> c b k", b=NNOD), wr_v)
            nc.vector.tensor_reduce(
                out1[:, b0 * 4:(b0 + NNOD) * 4].rearrange("c (b l) -> c b l", l=4),
                p_sb[:].rearrange("c (b l k) -> c b l k", l=4, k=16),
                axis=AX.X, op=mybir.AluOpType.add)

        prev = None
        for blk in range(NBLK):
            st = front(blk)
            if prev is not None:
                back(*prev)
            prev = (blk, st)
        back(*prev)

        # ---- lin + tail ----
        _tail(nc, tc, fpool, pp_misc, out1, l0, l1, sct, ident, OUT, F32)
    nc.compile()
    return nc


def _tail(nc, tc, fpool, pp_misc, out1, l0, l1, sct, ident, OUT, F32):
        import concourse.mybir as mybir
        o1v = out1[:].rearrange("c (b l) -> c b l", l=4)
        lo_ps = pp_misc.tile([C, BLOC], F32, tag="misc")
        nc.tensor.matmul(lo_ps[:], l0[:], o1v[:, :, 0], start=True, stop=True)
        l1_ps = pp_misc.tile([C, BLOC * 3], F32, tag="misc")
        nc.tensor.matmul(l1_ps[:].rearrange("f (b d) -> f b d", d=3), l1[:],
                         o1v[:, :, 1:4], start=True, stop=True)
        lo_sb = fpool.tile([C, BLOC], F32, tag="lo_sb")
        nc.vector.tensor_copy(lo_sb[:], lo_ps[:])
        l1_sb = fpool.tile([C, BLOC * 3], F32, tag="l1_sb")
        nc.vector.tensor_copy(l1_sb[:], l1_ps[:])
        outt = fpool.tile([BLOC, 512], F32, tag="outt")
        tps = pp_misc.tile([BLOC, C], F32, tag="misc")
        nc.tensor.transpose(tps[:], lo_sb[:], ident[:])
        nc.vector.tensor_add(outt[:, 0:128], tps[:], sct[:, 0:128])
        l1v = l1_sb[:].rearrange("f (b d) -> f b d", d=3)
        o_v = outt[:, 128:].rearrange("b (f d) -> b f d", d=3)
        s_v = sct[:, 128:].rearrange("b (f d) -> b f d", d=3)
        for ddi in range(3):
            tpd = pp_misc.tile([BLOC, C], F32, tag="misc")
            nc.tensor.transpose(tpd[:], l1v[:, :, ddi], ident[:])
            nc.vector.tensor_add(o_v[:, :, ddi], tpd[:], s_v[:, :, ddi])
        nc.sync.dma_start(OUT[:], outt[:])


_PROG = {}


def kernel(**inputs):
    import concourse.bass_utils as bass_utils

    consts = _build_consts(inputs)

    nf = np.asarray(inputs["node_feats"], np.float32)
    attrs = np.asarray(inputs["node_attrs"], np.float32)
    sc = np.asarray(inputs["sc"], np.float32)

    if "prog" not in _PROG:
        _PROG["prog"] = build_program()
    nc = _PROG["prog"]

    in_maps = []
    for r in range(NCORES):
        b0 = r * BLOC
        xt = nf[b0:b0 + BLOC].transpose(2, 0, 1).reshape(16, NLOC)
        # 3-lane pack: lane Lb at partition base 32*Lb holds column blocks
        # [Lb*LBLK, (Lb+1)*LBLK)
        x3 = np.zeros((80, LANEW), np.float32)
        for blk in range(NBLK):
            Lb, cb = blk // LBLK, blk % LBLK
            x3[32 * Lb:32 * Lb + 16, cb * NB:(cb + 1) * NB] = xt[:, blk * NB:(blk + 1) * NB]
        m = {"X_Tm": x3,
             "attrsT": np.ascontiguousarray(attrs[b0:b0 + BLOC].T),
             "sc": np.ascontiguousarray(sc[b0:b0 + BLOC])}
        m.update(consts)
        in_maps.append(m)

    res = bass_utils.run_bass_kernel_spmd(
        nc, in_maps, list(range(NCORES)),
        trace=os.environ.get("KTRACE", "0") == "1")
    global LAST_EXEC_NS
    LAST_EXEC_NS = getattr(res, "exec_time_ns", None)
    outs = [np.asarray(res.results[r]["OUT"]) for r in range(NCORES)]
    return np.concatenate(outs, axis=0).astype(np.float32)


LAST_EXEC_NS = None


# revision 52
# speedup vs baseline: 1.3195x; 1.0461x over previous
"""Trainium2 Bass kernel for nn_EquivariantProductBasisBlock (MACE product basis).

Per (node b, channel c) the block computes a symmetric cubic polynomial in
x = node_feats[b,c,:] (16-dim). v2 basis: powers of linear forms.

  quad monomials q_ij = x_i x_j   -> spanned by squares   (a2*(x_i+x_j))^2
  cubic monomials t_ijm           -> spanned by cubes     (a3*(x_i+x_j+x_m))^3

The change of basis (A2/A3, cond ~6/21) is folded into U on the host in fp64,
so on-chip each basis tile is: one PE selection matmul (ell = Sel @ x), one
ScalarE Square (PSUM->SBUF), and for cubics one DVE/Pool scalar_tensor_tensor
(ell * ell^2). Basis tiles are paired into [128,1024] two-bank PSUM supertiles
so one ScalarE Square covers two tiles.

  G[(ld,kap), n] = Ufold.T @ [x; squares; cubes]      -- 10 PE matmuls / block
  Wrep[c,(kap,b)] = WK.T @ attrs.T (bf16 PE)          -- exact for dense attrs
  out1[c,(b,ld)]  = sum_kap G'[c,(b,ld,kap)] * Wrep'  -- transpose + VE
  out[b] = concat_li(lin_li.T @ out1)/sqrt(C) + sc

Sharding: data-parallel over nodes, 128 nodes/core on 8 cores, no collectives.
"""
import math
import os
import numpy as np

N, C, L, E = 1024, 128, 16, 10
NCORES = 8
BLOC = N // NCORES            # nodes per core
NLOC = BLOC * C               # (b,c) columns per core; n = b*C + c
NB = 512                      # column block (one fp32 PSUM bank)
NBLK = NLOC // NB
NNOD = NB // C                # nodes per block
LBLK = (NBLK + 2) // 3        # column blocks per partition lane (X packing)
LANEW = LBLK * NB             # free width per lane

PAIRS = [(i, j) for j in range(L) for i in range(j + 1)]              # 136
TRIPLES = [(i, j, m) for j in range(L) for i in range(j + 1) for m in range(j, L)]
NQ, NT = len(PAIRS), len(TRIPLES)                                      # 136, 816

# 7 cubic basis tiles of <=128 rows (base partition 0, zero-padded).  The 136
# "special" triples (i,j,15) live in tiles 5/6; their Act-Square intermediates
# (ell^2) span all quadratic monomials, so the quad U-path folds into two
# extra G matmuls reading c2 of tiles 5/6 — no dedicated quadratic tiles.
NTILE = 7
NSLOT = 9                     # U_all slots: 7 cube + 2 quad (c2 of tiles 5,6)

A2S = 1.0 / math.sqrt(2.0)    # scale for quad linear forms
A3S = 1.0 / math.sqrt(3.0)    # scale for cubic linear forms


def _build_consts(inputs):
    import itertools
    f32 = np.float32
    Us = [{nu: np.asarray(inputs[f"U_{li}_{nu}"], np.float64) for nu in (1, 2, 3)}
          for li in range(2)]
    lins = [np.asarray(inputs[f"lin_{li}"], f32) for li in range(2)]

    row_of_pair = {p: r for r, p in enumerate(PAIRS)}
    row_of_triple = {}
    for r, (i, j, m) in enumerate(TRIPLES):
        row_of_triple[tuple(sorted((i, j, m)))] = r

    # base U coefficients on monomial bases (as in the reference contraction)
    UX = np.zeros((16, 64), np.float64)
    Uq = np.zeros((NQ, 64), np.float64)
    U3 = np.zeros((NT, 64), np.float64)
    for ld in range(4):
        li, dd = (0, 0) if ld == 0 else (1, ld - 1)
        U3t, U2t, U1t = Us[li][3], Us[li][2], Us[li][1]
        UX[:, ld * 16 + 15] = U1t[dd, :, 0]
        for r, (i, j) in enumerate(PAIRS):
            v = U2t[dd, i, j, :] + (U2t[dd, j, i, :] if i != j else 0.0)
            Uq[r, ld * 16 + 11:ld * 16 + 15] = v
        for r, (i, j, m) in enumerate(TRIPLES):
            if i < j < m:
                arr = [(i, j, m), (i, m, j), (j, i, m), (j, m, i), (m, i, j), (m, j, i)]
            elif i == j and j < m:
                arr = [(i, i, m), (i, m, i), (m, i, i)]
            elif i < j and j == m:
                arr = [(i, j, j), (j, i, j), (j, j, i)]
            else:
                arr = [(i, i, i)]
            U3[r, ld * 16:ld * 16 + 11] = sum(U3t[dd, a, b, c, :] for (a, b, c) in arr)

    # cubic change of basis: y3 = A3 t  (y3_r = (a3(x_i+x_j+x_m))^3)
    A3 = np.zeros((NT, NT))
    for r, (i, j, m) in enumerate(TRIPLES):
        for (u, v, w) in itertools.product((i, j, m), repeat=3):
            A3[r, row_of_triple[tuple(sorted((u, v, w)))]] += 1.0
    U3f = np.linalg.solve(A3.T * (A3S ** 3), U3)

    # quad fold: q monomials via (a3(x_i+x_j+x_15))^2 of the special triples
    B = np.zeros((NQ, NQ))
    for r, (i, j) in enumerate(PAIRS):
        cv = np.zeros(16)
        cv[i] += A3S; cv[j] += A3S; cv[15] += A3S
        for a in range(16):
            for b in range(a, 16):
                coef = cv[a] * cv[b] * (2.0 if a != b else 1.0)
                if coef:
                    B[r, row_of_pair[(a, b)]] += coef
    Vq = np.linalg.solve(B.T, Uq)                    # [136, 64] on special forms

    # triple ordering: OTHER (m != 15) first, then SPECIAL = (i,j,15) in PAIRS
    # order.  Tiles: 0..4 = other[128k:128(k+1)], 5 = other[640:680]+special[0:88],
    # 6 = special[88:136] (zero-padded)
    special_orig = [row_of_triple[tuple(sorted((i, j, 15)))] for (i, j) in PAIRS]
    other_orig = [r for r, t in enumerate(TRIPLES) if t[2] != 15]
    assert len(other_orig) == NT - NQ

    def tile_row(ti, p):
        """('o'|'s', idx) of the basis row at partition p of tile ti, or None"""
        if ti < 5:
            return ("o", ti * 128 + p)
        if ti == 5:
            return ("o", 640 + p) if p < 40 else ("s", p - 40)
        return ("s", 88 + p) if p < 48 else None

    SEL = np.zeros((16, NTILE * 128), np.float64)
    U_all = np.zeros((128, 64 * NSLOT), np.float64)
    for ti in range(NTILE):
        for p in range(128):
            r = tile_row(ti, p)
            if r is None:
                continue
            kind, k = r
            orig = other_orig[k] if kind == "o" else special_orig[k]
            i, j, m = TRIPLES[orig]
            SEL[i, ti * 128 + p] += A3S
            SEL[j, ti * 128 + p] += A3S
            SEL[m, ti * 128 + p] += A3S
            U_all[p, ti * 64:(ti + 1) * 64] = U3f[orig]
            if kind == "s":
                U_all[p, (7 + (ti - 5)) * 64:(8 + (ti - 5)) * 64] = Vq[k]

    # 3-lane packing at partition bases {0,32,64} (lhsT.base == rhs.base)
    def lane3(mat):
        rows = mat.shape[0]
        out = np.zeros((64 + rows, mat.shape[1]), mat.dtype)
        for Lb in range(3):
            out[32 * Lb:32 * Lb + rows] = mat
        return out

    # WK packed at 3 bases: kappa groups 0..23 | 24..47 | 48..63
    Ws = [{nu: np.asarray(inputs[f"W_{li}_{nu}"], f32) for nu in (1, 2, 3)}
          for li in range(2)]
    WKp = np.zeros((E, 64, C), f32)
    for ld in range(4):
        li = 0 if ld == 0 else 1
        WKp[:, ld * 16:ld * 16 + 11, :] = Ws[li][3]
        WKp[:, ld * 16 + 11:ld * 16 + 15, :] = Ws[li][2]
        WKp[:, ld * 16 + 15, :] = Ws[li][1][:, 0, :]
    WK3 = np.zeros((74, 24 * C), f32)
    for kap in range(64):
        g, off = (0, 0) if kap < 24 else ((1, 24) if kap < 48 else (2, 48))
        WK3[32 * g:32 * g + E, (kap - off) * C:(kap - off + 1) * C] = WKp[:, kap, :]

    isc = f32(1.0 / math.sqrt(C))
    return {
        "U_all": U_all.astype(f32),
        "UX3": lane3(UX.astype(f32)),
        "SEL3": lane3(SEL.astype(f32)),
        "WK3": WK3,
        "lin0": np.ascontiguousarray(lins[0] * isc),
        "lin1": np.ascontiguousarray(lins[1] * isc),
    }


def build_program():
    import concourse.bass as bass
    import concourse.bacc as bacc
    import concourse.mybir as mybir
    import concourse.tile as tile
    from concourse.masks import make_identity
    from contextlib import ExitStack

    dt = mybir.dt
    F32 = dt.float32
    F32R = dt.float32r
    BF16 = dt.bfloat16
    AX = mybir.AxisListType
    SQUARE = mybir.ActivationFunctionType.Square
    MULT = mybir.AluOpType.mult

    nc = bacc.Bacc(None, target_bir_lowering=False)
    X_Tm = nc.dram_tensor("X_Tm", [80, LANEW], F32, kind="ExternalInput")
    attrsT = nc.dram_tensor("attrsT", [E, BLOC], F32, kind="ExternalInput")
    sc_d = nc.dram_tensor("sc", [BLOC, 512], F32, kind="ExternalInput")
    U_all = nc.dram_tensor("U_all", [128, 64 * NSLOT], F32, kind="ExternalInput")
    UX3 = nc.dram_tensor("UX3", [80, 64], F32, kind="ExternalInput")
    SEL3 = nc.dram_tensor("SEL3", [80, NTILE * 128], F32, kind="ExternalInput")
    WK3 = nc.dram_tensor("WK3", [74, 24 * C], F32, kind="ExternalInput")
    lin0 = nc.dram_tensor("lin0", [C, C], F32, kind="ExternalInput")
    lin1 = nc.dram_tensor("lin1", [C, C], F32, kind="ExternalInput")
    OUT = nc.dram_tensor("OUT", [BLOC, 512], F32, kind="ExternalOutput")

    with tile.TileContext(nc) as tc, ExitStack() as ctx:
        cpool = ctx.enter_context(tc.tile_pool(name="consts", bufs=1))
        fpool = ctx.enter_context(tc.tile_pool(name="feats", bufs=2))
        spool = ctx.enter_context(tc.tile_pool(name="stream", bufs=8))
        dpool = ctx.enter_context(tc.tile_pool(name="dmab", bufs=4))
        # PSUM (8 banks): pair supertile (2 banks x 3 bufs) + g (1x1) + misc (1x1)
        pp_pair = ctx.enter_context(tc.tile_pool(name="ps_pair", bufs=3, space="PSUM"))
        pp_g = ctx.enter_context(tc.tile_pool(name="ps_g", bufs=1, space="PSUM"))
        pp_misc = ctx.enter_context(tc.tile_pool(name="ps_misc", bufs=1, space="PSUM"))

        # PE-consumed tiles laundered through one copy each so matmul operand
        # producers collapse onto a single engine.
        def launder(shape, dtp, tag, src):
            raw = cpool.tile(shape, src.dtype, tag=tag + "_r")
            nc.sync.dma_start(raw[:], src[:])
            t = cpool.tile(shape, dtp, tag=tag)
            nc.vector.tensor_copy(t[:], raw[:])
            return t

        xsm = launder([80, LANEW], F32R, "xTm", X_Tm)
        ua = launder([128, 64 * NSLOT], F32R, "uall", U_all)
        ux3 = launder([80, 64], F32R, "ux3", UX3)
        sel3 = launder([80, NTILE * 128], F32R, "sel3", SEL3)
        wk3 = launder([74, 24 * C], BF16, "wk3", WK3)
        l0 = launder([C, C], F32, "lin0", lin0)
        l1 = launder([C, C], F32, "lin1", lin1)
        # attrs replicated at the 3 bases to pair with WK3 lhsT slices
        ats_raw = cpool.tile([74, BLOC], F32, tag="attrs_r")
        ats = cpool.tile([74, BLOC], BF16, tag="attrs")
        for Lb in range(3):
            nc.sync.dma_start(ats_raw[32 * Lb:32 * Lb + E], attrsT[:])
            nc.vector.tensor_copy(ats[32 * Lb:32 * Lb + E], ats_raw[32 * Lb:32 * Lb + E])
        sct = cpool.tile([BLOC, 512], F32, tag="sc"); nc.sync.dma_start(sct[:], sc_d[:])
        ident_raw = cpool.tile([128, 128], F32, tag="ident_r")
        make_identity(nc, ident_raw[:])
        ident = cpool.tile([128, 128], F32, tag="ident")
        nc.vector.tensor_copy(ident[:], ident_raw[:])

        # Wrep' [c, (kap, b)]: 16 rounds x 4 kappa, K=10 bf16 matmuls.  Rounds
        # borrow the triple-buffered pair pool so matmuls pipeline against the
        # PSUM-exit copies, which alternate ScalarE/DVE.
        wrep = cpool.tile([C, 64 * BLOC], F32, tag="wrep")

        def wrep_round(rnd):
            wps = pp_pair.tile([128, 2 * NB], F32, tag="pair")
            for kk in range(4):
                kap = rnd * 4 + kk
                g3, off = (0, 0) if kap < 24 else ((1, 24) if kap < 48 else (2, 48))
                nc.tensor.matmul(
                    wps[:C, kk * BLOC:(kk + 1) * BLOC],
                    wk3[32 * g3:32 * g3 + E, (kap - off) * C:(kap - off + 1) * C],
                    ats[32 * g3:32 * g3 + E], start=True, stop=True)
            dst = wrep[:, rnd * 4 * BLOC:(rnd + 1) * 4 * BLOC]
            if rnd % 2 == 0:
                nc.scalar.copy(dst, wps[:C, 0:4 * BLOC])
            else:
                nc.vector.tensor_copy(dst, wps[:C, 0:4 * BLOC])

        out1 = cpool.tile([C, BLOC * 4], F32, tag="out1")  # [c, (b, ld)]
        dummy = cpool.tile([1, 8], F32, tag="dummy")

        # --- software-pipelined block loop: the basis front-end of block k
        # (sel matmuls, squares, cubes) is emitted BEFORE the G/out1 back-end
        # of block k-1, so the in-order PE stream never parks behind
        # dependent G matmuls while independent sel matmuls exist.
        def front(blk):
            Lb = blk // LBLK
            p0 = 32 * Lb
            csl = slice((blk % LBLK) * NB, (blk % LBLK + 1) * NB)
            xsm_b = xsm[p0:p0 + 16, csl]
            st = {"xsm_b": xsm_b, "p0": p0, "c2": {}, "t_sb": {}}
            # pairs of basis tiles in [128, 1024] two-bank PSUM supertiles:
            # (0,1) (2,3) (4,5) (6,-); one ScalarE Square per pair; cube STT
            # on DVE (pairs 0,3) or Pool via a PSUM->SBUF DMA bounce (1,2)
            for pi in (2, 0, 3, 1):
                tA, tB = 2 * pi, 2 * pi + 1
                nhalf = 1 if tB >= NTILE else 2
                w = nhalf * NB
                ps = pp_pair.tile([128, 2 * NB], F32, tag="pair")
                nc.tensor.matmul(ps[:, 0:NB], sel3[p0:p0 + 16, tA * 128:(tA + 1) * 128],
                                 xsm_b, start=True, stop=True)
                if nhalf == 2:
                    nc.tensor.matmul(ps[:, NB:2 * NB],
                                     sel3[p0:p0 + 16, tB * 128:(tB + 1) * 128],
                                     xsm_b, start=True, stop=True)
                c2 = spool.tile([128, 2 * NB], F32R, tag="c2")
                t_sb = spool.tile([128, 2 * NB], F32R, tag="t_sb")
                if pi == 2:
                    # Pool path (Pool cannot read PSUM): ScalarE bounces ell
                    # to SBUF, Pool squares and cubes it there
                    c_sb = dpool.tile([128, 2 * NB], F32, tag="c_sb")
                    nc.scalar.copy(c_sb[:, 0:w], ps[:, 0:w])
                    nc.gpsimd.tensor_mul(c2[:, 0:w], c_sb[:, 0:w], c_sb[:, 0:w])
                    nc.gpsimd.tensor_mul(t_sb[:, 0:w], c2[:, 0:w], c_sb[:, 0:w])
                else:
                    nc.scalar.activation(c2[:, 0:w], ps[:, 0:w], SQUARE)
                    nc.vector.scalar_tensor_tensor(
                        t_sb[:, 0:w], ps[:, 0:w], 1.0, c2[:, 0:w], MULT, MULT)
                st["c2"][pi] = c2
                st["t_sb"][pi] = t_sb
            return st

        def back(blk, st):
            g_ps = pp_g.tile([64, NB], F32, tag="g")
            p0 = st["p0"]
            nc.tensor.matmul(g_ps[:], ux3[p0:p0 + 16], st["xsm_b"],
                             start=True, stop=False)
            mms = []
            for pi in (2, 3, 0, 1):   # pool-pair operands ready last
                tA, tB = 2 * pi, 2 * pi + 1
                nhalf = 1 if tB >= NTILE else 2
                c2, t_sb = st["c2"][pi], st["t_sb"][pi]
                for h, ti in enumerate((tA, tB)[:nhalf]):
                    mms.append((ua[:, ti * 64:(ti + 1) * 64],
                                t_sb[:, h * NB:(h + 1) * NB]))
                    if ti >= 5:   # quad fold reads c2 of tiles 5/6
                        mms.append((ua[:, (2 + ti) * 64:(3 + ti) * 64],
                                    c2[:, h * NB:(h + 1) * NB]))
            for i, (lhsT, rhs) in enumerate(mms):
                nc.tensor.matmul(g_ps[:], lhsT, rhs, start=False,
                                 stop=i == len(mms) - 1)

            # ---- transpose G per node, mix with Wrep', reduce kappa ----
            g_sb = fpool.tile([64, NB], F32, tag="g_sb")
            nc.scalar.copy(g_sb[:], g_ps[:])
            gt_ps = pp_misc.tile([C, NNOD * 64], F32, tag="misc")
            for bb in range(NNOD):
                nc.tensor.transpose(gt_ps[:, bb * 64:(bb + 1) * 64],
                                    g_sb[:, bb * C:(bb + 1) * C], ident[:64, :64])
            b0 = blk * NNOD
            p_sb = fpool.tile([C, NNOD * 64], F32, tag="p_sb")
            wr_v = wrep[:].rearrange("c (k b) -> c b k", k=64)[:, b0:b0 + NNOD, :]
            nc.vector.tensor_mul(p_sb[:].rearrange("c (b k) -> c b k", b=NNOD),
                                 gt_ps[:].rearrange("c (b k) -> c b k", b=NNOD), wr_v)
            nc.vector.tensor_reduce(
                out1[:, b0 * 4:(b0 + NNOD) * 4].rearrange("c (b l) -> c b l", l=4),
                p_sb[:].rearrange("c (b l k) -> c b l k", l=4, k=16),
                axis=AX.X, op=mybir.AluOpType.add)

        prev = None
        for blk in range(NBLK):
            st = front(blk)
            if blk == 1:
                for rnd in range(16):
                    wrep_round(rnd)
            if prev is not None:
                back(*prev)
            if blk == 17:
                _tail(nc, tc, fpool, pp_misc, out1, l0, l1, sct, ident, OUT,
                      F32, 0, BLOC // 2)
            prev = (blk, st)
        back(*prev)

        # ---- lin + tail (second half) ----
        _tail(nc, tc, fpool, pp_misc, out1, l0, l1, sct, ident, OUT, F32,
              BLOC // 2, BLOC)
    nc.compile()
    return nc


def _tail(nc, tc, fpool, pp_misc, out1, l0, l1, sct, ident, OUT, F32, n0, n1):
        import concourse.mybir as mybir
        nh = n1 - n0
        o1v = out1[:].rearrange("c (b l) -> c b l", l=4)[:, n0:n1, :]
        lo_ps = pp_misc.tile([C, nh], F32, tag="misc")
        nc.tensor.matmul(lo_ps[:], l0[:], o1v[:, :, 0], start=True, stop=True)
        l1_ps = pp_misc.tile([C, nh * 3], F32, tag="misc")
        nc.tensor.matmul(l1_ps[:].rearrange("f (b d) -> f b d", d=3), l1[:],
                         o1v[:, :, 1:4], start=True, stop=True)
        lo_sb = fpool.tile([C, nh], F32, tag="lo_sb")
        nc.vector.tensor_copy(lo_sb[:], lo_ps[:])
        l1_sb = fpool.tile([C, nh * 3], F32, tag="l1_sb")
        nc.vector.tensor_copy(l1_sb[:], l1_ps[:])
        outt = fpool.tile([nh, 512], F32, tag="outt")
        tps = pp_misc.tile([nh, C], F32, tag="misc")
        nc.tensor.transpose(tps[:], lo_sb[:], ident[:])
        nc.vector.tensor_add(outt[:, 0:128], tps[:], sct[n0:n1, 0:128])
        l1v = l1_sb[:].rearrange("f (b d) -> f b d", d=3)
        o_v = outt[:, 128:].rearrange("b (f d) -> b f d", d=3)
        s_v = sct[n0:n1, 128:].rearrange("b (f d) -> b f d", d=3)
        for ddi in range(3):
            tpd = pp_misc.tile([nh, C], F32, tag="misc")
            nc.tensor.transpose(tpd[:], l1v[:, :, ddi], ident[:])
            nc.vector.tensor_add(o_v[:, :, ddi], tpd[:], s_v[:, :, ddi])
        nc.sync.dma_start(OUT[n0:n1], outt[:])


_PROG = {}


def kernel(**inputs):
    import concourse.bass_utils as bass_utils

    consts = _build_consts(inputs)

    nf = np.asarray(inputs["node_feats"], np.float32)
    attrs = np.asarray(inputs["node_attrs"], np.float32)
    sc = np.asarray(inputs["sc"], np.float32)

    if "prog" not in _PROG:
        _PROG["prog"] = build_program()
    nc = _PROG["prog"]

    in_maps = []
    for r in range(NCORES):
        b0 = r * BLOC
        xt = nf[b0:b0 + BLOC].transpose(2, 0, 1).reshape(16, NLOC)
        # 3-lane pack: lane Lb at partition base 32*Lb holds column blocks
        # [Lb*LBLK, (Lb+1)*LBLK)
        x3 = np.zeros((80, LANEW), np.float32)
        for blk in range(NBLK):
            Lb, cb = blk // LBLK, blk % LBLK
            x3[32 * Lb:32 * Lb + 16, cb * NB:(cb + 1) * NB] = xt[:, blk * NB:(blk + 1) * NB]
        m = {"X_Tm": x3,
             "attrsT": np.ascontiguousarray(attrs[b0:b0 + BLOC].T),
             "sc": np.ascontiguousarray(sc[b0:b0 + BLOC])}
        m.update(consts)
        in_maps.append(m)

    res = bass_utils.run_bass_kernel_spmd(
        nc, in_maps, list(range(NCORES)),
        trace=os.environ.get("KTRACE", "0") == "1")
    global LAST_EXEC_NS
    LAST_EXEC_NS = getattr(res, "exec_time_ns", None)
    outs = [np.asarray(res.results[r]["OUT"]) for r in range(NCORES)]
    return np.concatenate(outs, axis=0).astype(np.float32)


LAST_EXEC_NS = None


# revision 55
# speedup vs baseline: 1.4187x; 1.0752x over previous
"""Trainium2 Bass kernel for nn_EquivariantProductBasisBlock (MACE product basis).

Per (node b, channel c) the block computes a symmetric cubic polynomial in
x = node_feats[b,c,:] (16-dim). v2 basis: powers of linear forms.

  quad monomials q_ij = x_i x_j   -> spanned by squares   (a2*(x_i+x_j))^2
  cubic monomials t_ijm           -> spanned by cubes     (a3*(x_i+x_j+x_m))^3

The change of basis (A2/A3, cond ~6/21) is folded into U on the host in fp64,
so on-chip each basis tile is: one PE selection matmul (ell = Sel @ x), one
ScalarE Square (PSUM->SBUF), and for cubics one DVE/Pool scalar_tensor_tensor
(ell * ell^2). Basis tiles are paired into [128,1024] two-bank PSUM supertiles
so one ScalarE Square covers two tiles.

  G[(ld,kap), n] = Ufold.T @ [x; squares; cubes]      -- 10 PE matmuls / block
  Wrep[c,(kap,b)] = WK.T @ attrs.T (bf16 PE)          -- exact for dense attrs
  out1[c,(b,ld)]  = sum_kap G'[c,(b,ld,kap)] * Wrep'  -- transpose + VE
  out[b] = concat_li(lin_li.T @ out1)/sqrt(C) + sc

Sharding: data-parallel over nodes, 128 nodes/core on 8 cores, no collectives.
"""
import math
import os
import numpy as np

N, C, L, E = 1024, 128, 16, 10
NCORES = 8
BLOC = N // NCORES            # nodes per core
NLOC = BLOC * C               # (b,c) columns per core; n = b*C + c
NB = 512                      # column block (one fp32 PSUM bank)
NBLK = NLOC // NB
NNOD = NB // C                # nodes per block
LBLK = (NBLK + 2) // 3        # column blocks per partition lane (X packing)
LANEW = LBLK * NB             # free width per lane

PAIRS = [(i, j) for j in range(L) for i in range(j + 1)]              # 136
TRIPLES = [(i, j, m) for j in range(L) for i in range(j + 1) for m in range(j, L)]
NQ, NT = len(PAIRS), len(TRIPLES)                                      # 136, 816

# 7 cubic basis tiles of <=128 rows (base partition 0, zero-padded).  The 136
# "special" triples (i,j,15) live in tiles 5/6; their Act-Square intermediates
# (ell^2) span all quadratic monomials, so the quad U-path folds into two
# extra G matmuls reading c2 of tiles 5/6 — no dedicated quadratic tiles.
NTILE = 7
NSLOT = 9                     # U_all slots: 7 cube + 2 quad (c2 of tiles 5,6)

A2S = 1.0 / math.sqrt(2.0)    # scale for quad linear forms
A3S = 1.0 / math.sqrt(3.0)    # scale for cubic linear forms


def _build_consts(inputs):
    import itertools
    f32 = np.float32
    Us = [{nu: np.asarray(inputs[f"U_{li}_{nu}"], np.float64) for nu in (1, 2, 3)}
          for li in range(2)]
    lins = [np.asarray(inputs[f"lin_{li}"], f32) for li in range(2)]

    row_of_pair = {p: r for r, p in enumerate(PAIRS)}
    row_of_triple = {}
    for r, (i, j, m) in enumerate(TRIPLES):
        row_of_triple[tuple(sorted((i, j, m)))] = r

    # base U coefficients on monomial bases (as in the reference contraction)
    UX = np.zeros((16, 64), np.float64)
    Uq = np.zeros((NQ, 64), np.float64)
    U3 = np.zeros((NT, 64), np.float64)
    for ld in range(4):
        li, dd = (0, 0) if ld == 0 else (1, ld - 1)
        U3t, U2t, U1t = Us[li][3], Us[li][2], Us[li][1]
        UX[:, ld * 16 + 15] = U1t[dd, :, 0]
        for r, (i, j) in enumerate(PAIRS):
            v = U2t[dd, i, j, :] + (U2t[dd, j, i, :] if i != j else 0.0)
            Uq[r, ld * 16 + 11:ld * 16 + 15] = v
        for r, (i, j, m) in enumerate(TRIPLES):
            if i < j < m:
                arr = [(i, j, m), (i, m, j), (j, i, m), (j, m, i), (m, i, j), (m, j, i)]
            elif i == j and j < m:
                arr = [(i, i, m), (i, m, i), (m, i, i)]
            elif i < j and j == m:
                arr = [(i, j, j), (j, i, j), (j, j, i)]
            else:
                arr = [(i, i, i)]
            U3[r, ld * 16:ld * 16 + 11] = sum(U3t[dd, a, b, c, :] for (a, b, c) in arr)

    # cubic change of basis: y3 = A3 t  (y3_r = (a3(x_i+x_j+x_m))^3)
    A3 = np.zeros((NT, NT))
    for r, (i, j, m) in enumerate(TRIPLES):
        for (u, v, w) in itertools.product((i, j, m), repeat=3):
            A3[r, row_of_triple[tuple(sorted((u, v, w)))]] += 1.0
    U3f = np.linalg.solve(A3.T * (A3S ** 3), U3)

    # quad fold: q monomials via (a3(x_i+x_j+x_15))^2 of the special triples
    B = np.zeros((NQ, NQ))
    for r, (i, j) in enumerate(PAIRS):
        cv = np.zeros(16)
        cv[i] += A3S; cv[j] += A3S; cv[15] += A3S
        for a in range(16):
            for b in range(a, 16):
                coef = cv[a] * cv[b] * (2.0 if a != b else 1.0)
                if coef:
                    B[r, row_of_pair[(a, b)]] += coef
    Vq = np.linalg.solve(B.T, Uq)                    # [136, 64] on special forms

    # triple ordering: OTHER (m != 15) first, then SPECIAL = (i,j,15) in PAIRS
    # order.  Tiles: 0..4 = other[128k:128(k+1)], 5 = other[640:680]+special[0:88],
    # 6 = special[88:136] (zero-padded)
    special_orig = [row_of_triple[tuple(sorted((i, j, 15)))] for (i, j) in PAIRS]
    other_orig = [r for r, t in enumerate(TRIPLES) if t[2] != 15]
    assert len(other_orig) == NT - NQ

    def tile_row(ti, p):
        """('o'|'s', idx) of the basis row at partition p of tile ti, or None"""
        if ti < 5:
            return ("o", ti * 128 + p)
        if ti == 5:
            return ("o", 640 + p) if p < 40 else ("s", p - 40)
        return ("s", 88 + p) if p < 48 else None

    SEL = np.zeros((16, NTILE * 128), np.float64)
    U_all = np.zeros((128, 64 * NSLOT), np.float64)
    for ti in range(NTILE):
        for p in range(128):
            r = tile_row(ti, p)
            if r is None:
                continue
            kind, k = r
            orig = other_orig[k] if kind == "o" else special_orig[k]
            i, j, m = TRIPLES[orig]
            SEL[i, ti * 128 + p] += A3S
            SEL[j, ti * 128 + p] += A3S
            SEL[m, ti * 128 + p] += A3S
            U_all[p, ti * 64:(ti + 1) * 64] = U3f[orig]
            if kind == "s":
                U_all[p, (7 + (ti - 5)) * 64:(8 + (ti - 5)) * 64] = Vq[k]

    # 3-lane packing at partition bases {0,32,64} (lhsT.base == rhs.base)
    def lane3(mat):
        rows = mat.shape[0]
        out = np.zeros((64 + rows, mat.shape[1]), mat.dtype)
        for Lb in range(3):
            out[32 * Lb:32 * Lb + rows] = mat
        return out

    # WK packed at 3 bases: kappa groups 0..23 | 24..47 | 48..63
    Ws = [{nu: np.asarray(inputs[f"W_{li}_{nu}"], f32) for nu in (1, 2, 3)}
          for li in range(2)]
    WKp = np.zeros((E, 64, C), f32)
    for ld in range(4):
        li = 0 if ld == 0 else 1
        WKp[:, ld * 16:ld * 16 + 11, :] = Ws[li][3]
        WKp[:, ld * 16 + 11:ld * 16 + 15, :] = Ws[li][2]
        WKp[:, ld * 16 + 15, :] = Ws[li][1][:, 0, :]
    WK3 = np.zeros((74, 24 * C), f32)
    for kap in range(64):
        g, off = (0, 0) if kap < 24 else ((1, 24) if kap < 48 else (2, 48))
        WK3[32 * g:32 * g + E, (kap - off) * C:(kap - off + 1) * C] = WKp[:, kap, :]

    isc = f32(1.0 / math.sqrt(C))
    return {
        "U_all": U_all.astype(f32),
        "UX3": lane3(UX.astype(f32)),
        "SEL3": lane3(SEL.astype(f32)),
        "WK3": WK3,
        "lin0": np.ascontiguousarray(lins[0] * isc),
        "lin1": np.ascontiguousarray(lins[1] * isc),
    }


def build_program():
    import concourse.bass as bass
    import concourse.bacc as bacc
    import concourse.mybir as mybir
    import concourse.tile as tile
    from concourse.masks import make_identity
    from contextlib import ExitStack

    dt = mybir.dt
    F32 = dt.float32
    F32R = dt.float32r
    BF16 = dt.bfloat16
    AX = mybir.AxisListType
    SQUARE = mybir.ActivationFunctionType.Square
    MULT = mybir.AluOpType.mult

    nc = bacc.Bacc(None, target_bir_lowering=False)
    X_Tm = nc.dram_tensor("X_Tm", [80, LANEW], F32, kind="ExternalInput")
    attrsT = nc.dram_tensor("attrsT", [E, BLOC], F32, kind="ExternalInput")
    sc_d = nc.dram_tensor("sc", [BLOC, 512], F32, kind="ExternalInput")
    U_all = nc.dram_tensor("U_all", [128, 64 * NSLOT], F32, kind="ExternalInput")
    UX3 = nc.dram_tensor("UX3", [80, 64], F32, kind="ExternalInput")
    SEL3 = nc.dram_tensor("SEL3", [80, NTILE * 128], F32, kind="ExternalInput")
    WK3 = nc.dram_tensor("WK3", [74, 24 * C], F32, kind="ExternalInput")
    lin0 = nc.dram_tensor("lin0", [C, C], F32, kind="ExternalInput")
    lin1 = nc.dram_tensor("lin1", [C, C], F32, kind="ExternalInput")
    OUT = nc.dram_tensor("OUT", [BLOC, 512], F32, kind="ExternalOutput")

    with tile.TileContext(nc) as tc, ExitStack() as ctx:
        cpool = ctx.enter_context(tc.tile_pool(name="consts", bufs=1))
        fpool = ctx.enter_context(tc.tile_pool(name="feats", bufs=2))
        spool = ctx.enter_context(tc.tile_pool(name="stream", bufs=8))
        dpool = ctx.enter_context(tc.tile_pool(name="dmab", bufs=4))
        # PSUM (8 banks): pair supertile (2 banks x 3 bufs) + g (1x1) + misc (1x1)
        pp_pair = ctx.enter_context(tc.tile_pool(name="ps_pair", bufs=3, space="PSUM"))
        pp_g = ctx.enter_context(tc.tile_pool(name="ps_g", bufs=1, space="PSUM"))
        pp_misc = ctx.enter_context(tc.tile_pool(name="ps_misc", bufs=1, space="PSUM"))

        # PE-consumed tiles laundered through one copy each so matmul operand
        # producers collapse onto a single engine.
        def launder(shape, dtp, tag, src):
            raw = cpool.tile(shape, src.dtype, tag=tag + "_r")
            nc.sync.dma_start(raw[:], src[:])
            t = cpool.tile(shape, dtp, tag=tag)
            nc.vector.tensor_copy(t[:], raw[:])
            return t

        xsm = launder([80, LANEW], F32R, "xTm", X_Tm)
        ua = launder([128, 64 * NSLOT], F32R, "uall", U_all)
        ux3 = launder([80, 64], F32R, "ux3", UX3)
        sel3 = launder([80, NTILE * 128], F32R, "sel3", SEL3)
        wk3 = launder([74, 24 * C], BF16, "wk3", WK3)
        l0 = launder([C, C], F32, "lin0", lin0)
        l1 = launder([C, C], F32, "lin1", lin1)
        # attrs replicated at the 3 bases to pair with WK3 lhsT slices
        ats_raw = cpool.tile([74, BLOC], F32, tag="attrs_r")
        ats = cpool.tile([74, BLOC], BF16, tag="attrs")
        for Lb in range(3):
            nc.sync.dma_start(ats_raw[32 * Lb:32 * Lb + E], attrsT[:])
            nc.vector.tensor_copy(ats[32 * Lb:32 * Lb + E], ats_raw[32 * Lb:32 * Lb + E])
        sct = cpool.tile([BLOC, 512], F32, tag="sc"); nc.sync.dma_start(sct[:], sc_d[:])
        ident_raw = cpool.tile([128, 128], F32, tag="ident_r")
        make_identity(nc, ident_raw[:])
        ident = cpool.tile([128, 128], F32, tag="ident")
        nc.vector.tensor_copy(ident[:], ident_raw[:])

        # Wrep' [c, (kap, b)]: 16 rounds x 4 kappa, K=10 bf16 matmuls.  Rounds
        # borrow the triple-buffered pair pool so matmuls pipeline against the
        # PSUM-exit copies, which alternate ScalarE/DVE.
        wrep = cpool.tile([C, 64 * BLOC], F32, tag="wrep")

        def wrep_round(rnd):
            wps = pp_pair.tile([128, 2 * NB], F32, tag="pair")
            for kk in range(4):
                kap = rnd * 4 + kk
                g3, off = (0, 0) if kap < 24 else ((1, 24) if kap < 48 else (2, 48))
                nc.tensor.matmul(
                    wps[:C, kk * BLOC:(kk + 1) * BLOC],
                    wk3[32 * g3:32 * g3 + E, (kap - off) * C:(kap - off + 1) * C],
                    ats[32 * g3:32 * g3 + E], start=True, stop=True)
            dst = wrep[:, rnd * 4 * BLOC:(rnd + 1) * 4 * BLOC]
            if rnd % 2 == 0:
                nc.scalar.copy(dst, wps[:C, 0:4 * BLOC])
            else:
                nc.vector.tensor_copy(dst, wps[:C, 0:4 * BLOC])

        out1 = cpool.tile([C, BLOC * 4], F32, tag="out1")  # [c, (b, ld)]
        dummy = cpool.tile([1, 8], F32, tag="dummy")

        # --- software-pipelined block loop: the basis front-end of block k
        # (sel matmuls, squares, cubes) is emitted BEFORE the G/out1 back-end
        # of block k-1, so the in-order PE stream never parks behind
        # dependent G matmuls while independent sel matmuls exist.
        def front(blk):
            Lb = blk // LBLK
            p0 = 32 * Lb
            csl = slice((blk % LBLK) * NB, (blk % LBLK + 1) * NB)
            xsm_b = xsm[p0:p0 + 16, csl]
            st = {"xsm_b": xsm_b, "p0": p0, "c2": {}, "t_sb": {}}
            # pairs of basis tiles in [128, 1024] two-bank PSUM supertiles:
            # (0,1) (2,3) (4,5) (6,-); one ScalarE Square per pair; cube STT
            # on DVE (pairs 0,3) or Pool via a PSUM->SBUF DMA bounce (1,2)
            for pi in (2, 0, 3, 1):
                tA, tB = 2 * pi, 2 * pi + 1
                nhalf = 1 if tB >= NTILE else 2
                w = nhalf * NB
                ps = pp_pair.tile([128, 2 * NB], F32, tag="pair")
                nc.tensor.matmul(ps[:, 0:NB], sel3[p0:p0 + 16, tA * 128:(tA + 1) * 128],
                                 xsm_b, start=True, stop=True)
                if nhalf == 2:
                    nc.tensor.matmul(ps[:, NB:2 * NB],
                                     sel3[p0:p0 + 16, tB * 128:(tB + 1) * 128],
                                     xsm_b, start=True, stop=True)
                c2 = spool.tile([128, 2 * NB], F32R, tag="c2")
                t_sb = spool.tile([128, 2 * NB], F32R, tag="t_sb")
                if pi == 2:
                    # Pool path (Pool cannot read PSUM): ScalarE bounces ell
                    # to SBUF, Pool squares and cubes it there
                    c_sb = dpool.tile([128, 2 * NB], F32, tag="c_sb")
                    nc.scalar.copy(c_sb[:, 0:w], ps[:, 0:w])
                    nc.gpsimd.tensor_mul(c2[:, 0:w], c_sb[:, 0:w], c_sb[:, 0:w])
                    nc.gpsimd.tensor_mul(t_sb[:, 0:w], c2[:, 0:w], c_sb[:, 0:w])
                else:
                    nc.scalar.activation(c2[:, 0:w], ps[:, 0:w], SQUARE)
                    nc.vector.scalar_tensor_tensor(
                        t_sb[:, 0:w], ps[:, 0:w], 1.0, c2[:, 0:w], MULT, MULT)
                st["c2"][pi] = c2
                st["t_sb"][pi] = t_sb
            return st

        def back(blk, st):
            g_ps = pp_g.tile([64, NB], F32, tag="g")
            p0 = st["p0"]
            nc.tensor.matmul(g_ps[:], ux3[p0:p0 + 16], st["xsm_b"],
                             start=True, stop=False)
            mms = []
            for pi in (3, 0, 1, 2):   # pool-pair operands ready last
                tA, tB = 2 * pi, 2 * pi + 1
                nhalf = 1 if tB >= NTILE else 2
                c2, t_sb = st["c2"][pi], st["t_sb"][pi]
                for h, ti in enumerate((tA, tB)[:nhalf]):
                    mms.append((ua[:, ti * 64:(ti + 1) * 64],
                                t_sb[:, h * NB:(h + 1) * NB]))
                    if ti >= 5:   # quad fold reads c2 of tiles 5/6
                        mms.append((ua[:, (2 + ti) * 64:(3 + ti) * 64],
                                    c2[:, h * NB:(h + 1) * NB]))
            for i, (lhsT, rhs) in enumerate(mms):
                nc.tensor.matmul(g_ps[:], lhsT, rhs, start=False,
                                 stop=i == len(mms) - 1)

            # ---- transpose G per node, mix with Wrep', reduce kappa ----
            g_sb = fpool.tile([64, NB], F32, tag="g_sb")
            nc.scalar.copy(g_sb[:], g_ps[:])
            gt_ps = pp_misc.tile([C, NNOD * 64], F32, tag="misc")
            for bb in range(NNOD):
                nc.tensor.transpose(gt_ps[:, bb * 64:(bb + 1) * 64],
                                    g_sb[:, bb * C:(bb + 1) * C], ident[:64, :64])
            b0 = blk * NNOD
            p_sb = fpool.tile([C, NNOD * 64], F32, tag="p_sb")
            wr_v = wrep[:].rearrange("c (k b) -> c b k", k=64)[:, b0:b0 + NNOD, :]
            nc.vector.tensor_mul(p_sb[:].rearrange("c (b k) -> c b k", b=NNOD),
                                 gt_ps[:].rearrange("c (b k) -> c b k", b=NNOD), wr_v)
            nc.vector.tensor_reduce(
                out1[:, b0 * 4:(b0 + NNOD) * 4].rearrange("c (b l) -> c b l", l=4),
                p_sb[:].rearrange("c (b l k) -> c b l k", l=4, k=16),
                axis=AX.X, op=mybir.AluOpType.add)

        prev = None
        for blk in range(NBLK):
            st = front(blk)
            if blk == 1:
                for rnd in range(16):
                    wrep_round(rnd)
            if prev is not None:
                back(*prev)
            if blk == 17:
                _tail(nc, tc, fpool, pp_misc, out1, l0, l1, sct, ident, OUT,
                      F32, 0, BLOC // 2)
            prev = (blk, st)
        back(*prev)

        # ---- lin + tail (second half) ----
        _tail(nc, tc, fpool, pp_misc, out1, l0, l1, sct, ident, OUT, F32,
              BLOC // 2, BLOC)
    nc.compile()
    return nc


def _tail(nc, tc, fpool, pp_misc, out1, l0, l1, sct, ident, OUT, F32, n0, n1):
        import concourse.mybir as mybir
        nh = n1 - n0
        o1v = out1[:].rearrange("c (b l) -> c b l", l=4)[:, n0:n1, :]
        lo_ps = pp_misc.tile([C, nh], F32, tag="misc")
        nc.tensor.matmul(lo_ps[:], l0[:], o1v[:, :, 0], start=True, stop=True)
        l1_ps = pp_misc.tile([C, nh * 3], F32, tag="misc")
        nc.tensor.matmul(l1_ps[:].rearrange("f (b d) -> f b d", d=3), l1[:],
                         o1v[:, :, 1:4], start=True, stop=True)
        lo_sb = fpool.tile([C, nh], F32, tag="lo_sb")
        nc.vector.tensor_copy(lo_sb[:], lo_ps[:])
        l1_sb = fpool.tile([C, nh * 3], F32, tag="l1_sb")
        nc.vector.tensor_copy(l1_sb[:], l1_ps[:])
        outt = fpool.tile([nh, 512], F32, tag="outt")
        tps = pp_misc.tile([nh, C], F32, tag="misc")
        nc.tensor.transpose(tps[:], lo_sb[:], ident[:])
        nc.vector.tensor_add(outt[:, 0:128], tps[:], sct[n0:n1, 0:128])
        l1v = l1_sb[:].rearrange("f (b d) -> f b d", d=3)
        o_v = outt[:, 128:].rearrange("b (f d) -> b f d", d=3)
        s_v = sct[n0:n1, 128:].rearrange("b (f d) -> b f d", d=3)
        for ddi in range(3):
            tpd = pp_misc.tile([nh, C], F32, tag="misc")
            nc.tensor.transpose(tpd[:], l1v[:, :, ddi], ident[:])
            nc.vector.tensor_add(o_v[:, :, ddi], tpd[:], s_v[:, :, ddi])
        nc.sync.dma_start(OUT[n0:n1], outt[:])


_PROG = {}


def kernel(**inputs):
    import concourse.bass_utils as bass_utils

    consts = _build_consts(inputs)

    nf = np.asarray(inputs["node_feats"], np.float32)
    attrs = np.asarray(inputs["node_attrs"], np.float32)
    sc = np.asarray(inputs["sc"], np.float32)

    if "prog" not in _PROG:
        _PROG["prog"] = build_program()
    nc = _PROG["prog"]

    in_maps = []
    for r in range(NCORES):
        b0 = r * BLOC
        xt = nf[b0:b0 + BLOC].transpose(2, 0, 1).reshape(16, NLOC)
        # 3-lane pack: lane Lb at partition base 32*Lb holds column blocks
        # [Lb*LBLK, (Lb+1)*LBLK)
        x3 = np.zeros((80, LANEW), np.float32)
        for blk in range(NBLK):
            Lb, cb = blk // LBLK, blk % LBLK
            x3[32 * Lb:32 * Lb + 16, cb * NB:(cb + 1) * NB] = xt[:, blk * NB:(blk + 1) * NB]
        m = {"X_Tm": x3,
             "attrsT": np.ascontiguousarray(attrs[b0:b0 + BLOC].T),
             "sc": np.ascontiguousarray(sc[b0:b0 + BLOC])}
        m.update(consts)
        in_maps.append(m)

    res = bass_utils.run_bass_kernel_spmd(
        nc, in_maps, list(range(NCORES)),
        trace=os.environ.get("KTRACE", "0") == "1")
    global LAST_EXEC_NS
    LAST_EXEC_NS = getattr(res, "exec_time_ns", None)
    outs = [np.asarray(res.results[r]["OUT"]) for r in range(NCORES)]
    return np.concatenate(outs, axis=0).astype(np.float32)


LAST_EXEC_NS = None


# revision 68
# speedup vs baseline: 1.4190x; 1.0002x over previous
"""Trainium2 Bass kernel for nn_EquivariantProductBasisBlock (MACE product basis).

Per (node b, channel c) the block computes a symmetric cubic polynomial in
x = node_feats[b,c,:] (16-dim). v2 basis: powers of linear forms.

  quad monomials q_ij = x_i x_j   -> spanned by squares   (a2*(x_i+x_j))^2
  cubic monomials t_ijm           -> spanned by cubes     (a3*(x_i+x_j+x_m))^3

The change of basis (A2/A3, cond ~6/21) is folded into U on the host in fp64,
so on-chip each basis tile is: one PE selection matmul (ell = Sel @ x), one
ScalarE Square (PSUM->SBUF), and for cubics one DVE/Pool scalar_tensor_tensor
(ell * ell^2). Basis tiles are paired into [128,1024] two-bank PSUM supertiles
so one ScalarE Square covers two tiles.

  G[(ld,kap), n] = Ufold.T @ [x; squares; cubes]      -- 10 PE matmuls / block
  Wrep[c,(kap,b)] = WK.T @ attrs.T (bf16 PE)          -- exact for dense attrs
  out1[c,(b,ld)]  = sum_kap G'[c,(b,ld,kap)] * Wrep'  -- transpose + VE
  out[b] = concat_li(lin_li.T @ out1)/sqrt(C) + sc

Sharding: data-parallel over nodes, 128 nodes/core on 8 cores, no collectives.
"""
import math
import os
import numpy as np

N, C, L, E = 1024, 128, 16, 10
NCORES = 8
BLOC = N // NCORES            # nodes per core
NLOC = BLOC * C               # (b,c) columns per core; n = b*C + c
NB = 512                      # column block (one fp32 PSUM bank)
NBLK = NLOC // NB
NNOD = NB // C                # nodes per block
LBLK = (NBLK + 2) // 3        # column blocks per partition lane (X packing)
LANEW = LBLK * NB             # free width per lane

PAIRS = [(i, j) for j in range(L) for i in range(j + 1)]              # 136
TRIPLES = [(i, j, m) for j in range(L) for i in range(j + 1) for m in range(j, L)]
NQ, NT = len(PAIRS), len(TRIPLES)                                      # 136, 816

# 7 cubic basis tiles of <=128 rows (base partition 0, zero-padded).  The 136
# "special" triples (i,j,15) live in tiles 5/6; their Act-Square intermediates
# (ell^2) span all quadratic monomials, so the quad U-path folds into two
# extra G matmuls reading c2 of tiles 5/6 — no dedicated quadratic tiles.
NTILE = 7
NSLOT = 9                     # U_all slots: 7 cube + 2 quad (c2 of tiles 5,6)

A2S = 1.0 / math.sqrt(2.0)    # scale for quad linear forms
A3S = 1.0 / math.sqrt(3.0)    # scale for cubic linear forms


def _build_consts(inputs):
    import itertools
    f32 = np.float32
    Us = [{nu: np.asarray(inputs[f"U_{li}_{nu}"], np.float64) for nu in (1, 2, 3)}
          for li in range(2)]
    lins = [np.asarray(inputs[f"lin_{li}"], f32) for li in range(2)]

    row_of_pair = {p: r for r, p in enumerate(PAIRS)}
    row_of_triple = {}
    for r, (i, j, m) in enumerate(TRIPLES):
        row_of_triple[tuple(sorted((i, j, m)))] = r

    # base U coefficients on monomial bases (as in the reference contraction)
    UX = np.zeros((16, 64), np.float64)
    Uq = np.zeros((NQ, 64), np.float64)
    U3 = np.zeros((NT, 64), np.float64)
    for ld in range(4):
        li, dd = (0, 0) if ld == 0 else (1, ld - 1)
        U3t, U2t, U1t = Us[li][3], Us[li][2], Us[li][1]
        UX[:, ld * 16 + 15] = U1t[dd, :, 0]
        for r, (i, j) in enumerate(PAIRS):
            v = U2t[dd, i, j, :] + (U2t[dd, j, i, :] if i != j else 0.0)
            Uq[r, ld * 16 + 11:ld * 16 + 15] = v
        for r, (i, j, m) in enumerate(TRIPLES):
            if i < j < m:
                arr = [(i, j, m), (i, m, j), (j, i, m), (j, m, i), (m, i, j), (m, j, i)]
            elif i == j and j < m:
                arr = [(i, i, m), (i, m, i), (m, i, i)]
            elif i < j and j == m:
                arr = [(i, j, j), (j, i, j), (j, j, i)]
            else:
                arr = [(i, i, i)]
            U3[r, ld * 16:ld * 16 + 11] = sum(U3t[dd, a, b, c, :] for (a, b, c) in arr)

    # cubic change of basis: y3 = A3 t  (y3_r = (a3(x_i+x_j+x_m))^3)
    A3 = np.zeros((NT, NT))
    for r, (i, j, m) in enumerate(TRIPLES):
        for (u, v, w) in itertools.product((i, j, m), repeat=3):
            A3[r, row_of_triple[tuple(sorted((u, v, w)))]] += 1.0
    U3f = np.linalg.solve(A3.T * (A3S ** 3), U3)

    # quad fold: q monomials via (a3(x_i+x_j+x_15))^2 of the special triples
    B = np.zeros((NQ, NQ))
    for r, (i, j) in enumerate(PAIRS):
        cv = np.zeros(16)
        cv[i] += A3S; cv[j] += A3S; cv[15] += A3S
        for a in range(16):
            for b in range(a, 16):
                coef = cv[a] * cv[b] * (2.0 if a != b else 1.0)
                if coef:
                    B[r, row_of_pair[(a, b)]] += coef
    Vq = np.linalg.solve(B.T, Uq)                    # [136, 64] on special forms

    # triple ordering: OTHER (m != 15) first, then SPECIAL = (i,j,15) in PAIRS
    # order.  Tiles: 0..4 = other[128k:128(k+1)], 5 = other[640:680]+special[0:88],
    # 6 = special[88:136] (zero-padded)
    special_orig = [row_of_triple[tuple(sorted((i, j, 15)))] for (i, j) in PAIRS]
    other_orig = [r for r, t in enumerate(TRIPLES) if t[2] != 15]
    assert len(other_orig) == NT - NQ

    def tile_row(ti, p):
        """('o'|'s', idx) of the basis row at partition p of tile ti, or None"""
        if ti < 5:
            return ("o", ti * 128 + p)
        if ti == 5:
            return ("o", 640 + p) if p < 40 else ("s", p - 40)
        return ("s", 88 + p) if p < 48 else None

    SEL = np.zeros((16, NTILE * 128), np.float64)
    U_all = np.zeros((128, 64 * NSLOT), np.float64)
    for ti in range(NTILE):
        for p in range(128):
            r = tile_row(ti, p)
            if r is None:
                continue
            kind, k = r
            orig = other_orig[k] if kind == "o" else special_orig[k]
            i, j, m = TRIPLES[orig]
            SEL[i, ti * 128 + p] += A3S
            SEL[j, ti * 128 + p] += A3S
            SEL[m, ti * 128 + p] += A3S
            U_all[p, ti * 64:(ti + 1) * 64] = U3f[orig]
            if kind == "s":
                U_all[p, (7 + (ti - 5)) * 64:(8 + (ti - 5)) * 64] = Vq[k]

    # 3-lane packing at partition bases {0,32,64} (lhsT.base == rhs.base)
    def lane3(mat):
        rows = mat.shape[0]
        out = np.zeros((64 + rows, mat.shape[1]), mat.dtype)
        for Lb in range(3):
            out[32 * Lb:32 * Lb + rows] = mat
        return out

    # WK packed at 3 bases: kappa groups 0..23 | 24..47 | 48..63
    Ws = [{nu: np.asarray(inputs[f"W_{li}_{nu}"], f32) for nu in (1, 2, 3)}
          for li in range(2)]
    WKp = np.zeros((E, 64, C), f32)
    for ld in range(4):
        li = 0 if ld == 0 else 1
        WKp[:, ld * 16:ld * 16 + 11, :] = Ws[li][3]
        WKp[:, ld * 16 + 11:ld * 16 + 15, :] = Ws[li][2]
        WKp[:, ld * 16 + 15, :] = Ws[li][1][:, 0, :]
    WK3 = np.zeros((74, 24 * C), f32)
    for kap in range(64):
        g, off = (0, 0) if kap < 24 else ((1, 24) if kap < 48 else (2, 48))
        WK3[32 * g:32 * g + E, (kap - off) * C:(kap - off + 1) * C] = WKp[:, kap, :]

    isc = f32(1.0 / math.sqrt(C))
    return {
        "U_all": U_all.astype(f32),
        "UX3": lane3(UX.astype(f32)),
        "SEL3": lane3(SEL.astype(f32)),
        "WK3": WK3,
        "lin0": np.ascontiguousarray(lins[0] * isc),
        "lin1": np.ascontiguousarray(lins[1] * isc),
    }


def build_program():
    import concourse.bass as bass
    import concourse.bacc as bacc
    import concourse.mybir as mybir
    import concourse.tile as tile
    from concourse.masks import make_identity
    from contextlib import ExitStack

    dt = mybir.dt
    F32 = dt.float32
    F32R = dt.float32r
    BF16 = dt.bfloat16
    AX = mybir.AxisListType
    SQUARE = mybir.ActivationFunctionType.Square
    MULT = mybir.AluOpType.mult

    nc = bacc.Bacc(None, target_bir_lowering=False)
    X_Tm = nc.dram_tensor("X_Tm", [80, LANEW], F32, kind="ExternalInput")
    attrsT = nc.dram_tensor("attrsT", [E, BLOC], F32, kind="ExternalInput")
    sc_d = nc.dram_tensor("sc", [BLOC, 512], F32, kind="ExternalInput")
    U_all = nc.dram_tensor("U_all", [128, 64 * NSLOT], F32, kind="ExternalInput")
    UX3 = nc.dram_tensor("UX3", [80, 64], F32, kind="ExternalInput")
    SEL3 = nc.dram_tensor("SEL3", [80, NTILE * 128], F32, kind="ExternalInput")
    WK3 = nc.dram_tensor("WK3", [74, 24 * C], F32, kind="ExternalInput")
    lin0 = nc.dram_tensor("lin0", [C, C], F32, kind="ExternalInput")
    lin1 = nc.dram_tensor("lin1", [C, C], F32, kind="ExternalInput")
    OUT = nc.dram_tensor("OUT", [BLOC, 512], F32, kind="ExternalOutput")

    with tile.TileContext(nc) as tc, ExitStack() as ctx:
        cpool = ctx.enter_context(tc.tile_pool(name="consts", bufs=1))
        fpool = ctx.enter_context(tc.tile_pool(name="feats", bufs=2))
        spool = ctx.enter_context(tc.tile_pool(name="stream", bufs=8))
        dpool = ctx.enter_context(tc.tile_pool(name="dmab", bufs=4))
        # PSUM (8 banks): pair supertile (2 banks x 3 bufs) + g (1x1) + misc (1x1)
        pp_pair = ctx.enter_context(tc.tile_pool(name="ps_pair", bufs=3, space="PSUM"))
        pp_g = ctx.enter_context(tc.tile_pool(name="ps_g", bufs=1, space="PSUM"))
        pp_misc = ctx.enter_context(tc.tile_pool(name="ps_misc", bufs=1, space="PSUM"))

        # PE-consumed tiles laundered through one copy each so matmul operand
        # producers collapse onto a single engine.
        def launder(shape, dtp, tag, src):
            raw = cpool.tile(shape, src.dtype, tag=tag + "_r")
            nc.sync.dma_start(raw[:], src[:])
            t = cpool.tile(shape, dtp, tag=tag)
            nc.vector.tensor_copy(t[:], raw[:])
            return t

        xsm = launder([80, LANEW], F32R, "xTm", X_Tm)
        ua = launder([128, 64 * NSLOT], F32R, "uall", U_all)
        ux3 = launder([80, 64], F32R, "ux3", UX3)
        sel3 = launder([80, NTILE * 128], F32R, "sel3", SEL3)
        wk3 = launder([74, 24 * C], BF16, "wk3", WK3)
        l0 = launder([C, C], F32, "lin0", lin0)
        l1 = launder([C, C], F32, "lin1", lin1)
        # attrs replicated at the 3 bases to pair with WK3 lhsT slices
        ats_raw = cpool.tile([74, BLOC], F32, tag="attrs_r")
        ats = cpool.tile([74, BLOC], BF16, tag="attrs")
        for Lb in range(3):
            nc.sync.dma_start(ats_raw[32 * Lb:32 * Lb + E], attrsT[:])
            nc.vector.tensor_copy(ats[32 * Lb:32 * Lb + E], ats_raw[32 * Lb:32 * Lb + E])
        sct = cpool.tile([BLOC, 512], F32, tag="sc"); nc.sync.dma_start(sct[:], sc_d[:])
        ident_raw = cpool.tile([128, 128], F32, tag="ident_r")
        make_identity(nc, ident_raw[:])
        ident = cpool.tile([128, 128], F32, tag="ident")
        nc.vector.tensor_copy(ident[:], ident_raw[:])

        # Wrep' [c, (kap, b)]: 16 rounds x 4 kappa, K=10 bf16 matmuls.  Rounds
        # borrow the triple-buffered pair pool so matmuls pipeline against the
        # PSUM-exit copies, which alternate ScalarE/DVE.
        wrep = cpool.tile([C, 64 * BLOC], F32, tag="wrep")

        def wrep_round(rnd):
            wps = pp_pair.tile([128, 2 * NB], F32, tag="pair")
            for kk in range(4):
                kap = rnd * 4 + kk
                g3, off = (0, 0) if kap < 24 else ((1, 24) if kap < 48 else (2, 48))
                nc.tensor.matmul(
                    wps[:C, kk * BLOC:(kk + 1) * BLOC],
                    wk3[32 * g3:32 * g3 + E, (kap - off) * C:(kap - off + 1) * C],
                    ats[32 * g3:32 * g3 + E], start=True, stop=True)
            dst = wrep[:, rnd * 4 * BLOC:(rnd + 1) * 4 * BLOC]
            if rnd % 2 == 0:
                nc.scalar.copy(dst, wps[:C, 0:4 * BLOC])
            else:
                nc.vector.tensor_copy(dst, wps[:C, 0:4 * BLOC])

        out1 = cpool.tile([C, BLOC * 4], F32, tag="out1")  # [c, (b, ld)]
        dummy = cpool.tile([1, 8], F32, tag="dummy")

        # --- software-pipelined block loop: the basis front-end of block k
        # (sel matmuls, squares, cubes) is emitted BEFORE the G/out1 back-end
        # of block k-1, so the in-order PE stream never parks behind
        # dependent G matmuls while independent sel matmuls exist.
        def front(blk):
            Lb = blk // LBLK
            p0 = 32 * Lb
            csl = slice((blk % LBLK) * NB, (blk % LBLK + 1) * NB)
            xsm_b = xsm[p0:p0 + 16, csl]
            st = {"xsm_b": xsm_b, "p0": p0, "c2": {}, "t_sb": {}}
            # pairs of basis tiles in [128, 1024] two-bank PSUM supertiles:
            # (0,1) (2,3) (4,5) (6,-); one ScalarE Square per pair; cube STT
            # on DVE (pairs 0,3) or Pool via a PSUM->SBUF DMA bounce (1,2)
            for pi in (2, 0, 3, 1):
                tA, tB = 2 * pi, 2 * pi + 1
                nhalf = 1 if tB >= NTILE else 2
                w = nhalf * NB
                ps = pp_pair.tile([128, 2 * NB], F32, tag="pair")
                nc.tensor.matmul(ps[:, 0:NB], sel3[p0:p0 + 16, tA * 128:(tA + 1) * 128],
                                 xsm_b, start=True, stop=True)
                if nhalf == 2:
                    nc.tensor.matmul(ps[:, NB:2 * NB],
                                     sel3[p0:p0 + 16, tB * 128:(tB + 1) * 128],
                                     xsm_b, start=True, stop=True)
                c2 = spool.tile([128, 2 * NB], F32R, tag="c2")
                t_sb = spool.tile([128, 2 * NB], F32R, tag="t_sb")
                if pi == 2:
                    # Pool path (Pool cannot read PSUM): ScalarE bounces ell
                    # to SBUF, Pool squares and cubes it there
                    c_sb = dpool.tile([128, 2 * NB], F32, tag="c_sb")
                    nc.scalar.copy(c_sb[:, 0:w], ps[:, 0:w])
                    nc.gpsimd.tensor_mul(c2[:, 0:w], c_sb[:, 0:w], c_sb[:, 0:w])
                    nc.gpsimd.tensor_mul(t_sb[:, 0:w], c2[:, 0:w], c_sb[:, 0:w])
                else:
                    nc.scalar.activation(c2[:, 0:w], ps[:, 0:w], SQUARE)
                    nc.vector.scalar_tensor_tensor(
                        t_sb[:, 0:w], ps[:, 0:w], 1.0, c2[:, 0:w], MULT, MULT)
                st["c2"][pi] = c2
                st["t_sb"][pi] = t_sb
            return st

        def back(blk, st):
            g_ps = pp_g.tile([64, NB], F32, tag="g")
            p0 = st["p0"]
            nc.tensor.matmul(g_ps[:], ux3[p0:p0 + 16], st["xsm_b"],
                             start=True, stop=False)
            mms = []
            for pi in (3, 0, 1, 2):   # pool-pair operands ready last
                tA, tB = 2 * pi, 2 * pi + 1
                nhalf = 1 if tB >= NTILE else 2
                c2, t_sb = st["c2"][pi], st["t_sb"][pi]
                for h, ti in enumerate((tA, tB)[:nhalf]):
                    mms.append((ua[:, ti * 64:(ti + 1) * 64],
                                t_sb[:, h * NB:(h + 1) * NB]))
                    if ti >= 5:   # quad fold reads c2 of tiles 5/6
                        mms.append((ua[:, (2 + ti) * 64:(3 + ti) * 64],
                                    c2[:, h * NB:(h + 1) * NB]))
            for i, (lhsT, rhs) in enumerate(mms):
                nc.tensor.matmul(g_ps[:], lhsT, rhs, start=False,
                                 stop=i == len(mms) - 1)

            # ---- transpose G per node, mix with Wrep', reduce kappa ----
            g_sb = fpool.tile([64, NB], F32, tag="g_sb")
            nc.scalar.copy(g_sb[:], g_ps[:])
            gt_ps = pp_misc.tile([C, NNOD * 64], F32, tag="misc")
            for bb in range(NNOD):
                nc.tensor.transpose(gt_ps[:, bb * 64:(bb + 1) * 64],
                                    g_sb[:, bb * C:(bb + 1) * C], ident[:64, :64])
            b0 = blk * NNOD
            p_sb = fpool.tile([C, NNOD * 64], F32, tag="p_sb")
            wr_v = wrep[:].rearrange("c (k b) -> c b k", k=64)[:, b0:b0 + NNOD, :]
            nc.vector.tensor_mul(p_sb[:].rearrange("c (b k) -> c b k", b=NNOD),
                                 gt_ps[:].rearrange("c (b k) -> c b k", b=NNOD), wr_v)
            nc.vector.tensor_reduce(
                out1[:, b0 * 4:(b0 + NNOD) * 4].rearrange("c (b l) -> c b l", l=4),
                p_sb[:].rearrange("c (b l k) -> c b l k", l=4, k=16),
                axis=AX.X, op=mybir.AluOpType.add)

        prev = None
        for blk in range(NBLK):
            st = front(blk)
            if blk == 1:
                for rnd in range(16):
                    wrep_round(rnd)
            if prev is not None:
                back(*prev)
            if blk == 17:
                _tail(nc, tc, fpool, pp_misc, out1, l0, l1, sct, ident, OUT,
                      F32, 0, BLOC // 2)
            if blk == 25:
                _tail(nc, tc, fpool, pp_misc, out1, l0, l1, sct, ident, OUT,
                      F32, BLOC // 2, 3 * BLOC // 4)
            prev = (blk, st)
        back(*prev)

        # ---- lin + tail (last quarter) ----
        _tail(nc, tc, fpool, pp_misc, out1, l0, l1, sct, ident, OUT, F32,
              3 * BLOC // 4, BLOC)
    nc.compile()
    return nc


def _tail(nc, tc, fpool, pp_misc, out1, l0, l1, sct, ident, OUT, F32, n0, n1):
        import concourse.mybir as mybir
        nh = n1 - n0
        o1v = out1[:].rearrange("c (b l) -> c b l", l=4)[:, n0:n1, :]
        lo_ps = pp_misc.tile([C, nh], F32, tag="misc")
        nc.tensor.matmul(lo_ps[:], l0[:], o1v[:, :, 0], start=True, stop=True)
        l1_ps = pp_misc.tile([C, nh * 3], F32, tag="misc")
        nc.tensor.matmul(l1_ps[:].rearrange("f (b d) -> f b d", d=3), l1[:],
                         o1v[:, :, 1:4], start=True, stop=True)
        lo_sb = fpool.tile([C, nh], F32, tag="lo_sb")
        nc.vector.tensor_copy(lo_sb[:], lo_ps[:])
        l1_sb = fpool.tile([C, nh * 3], F32, tag="l1_sb")
        nc.vector.tensor_copy(l1_sb[:], l1_ps[:])
        outt = fpool.tile([nh, 512], F32, tag="outt")
        tps = pp_misc.tile([nh, C], F32, tag="misc")
        nc.tensor.transpose(tps[:], lo_sb[:], ident[:])
        nc.vector.tensor_add(outt[:, 0:128], tps[:], sct[n0:n1, 0:128])
        l1v = l1_sb[:].rearrange("f (b d) -> f b d", d=3)
        o_v = outt[:, 128:].rearrange("b (f d) -> b f d", d=3)
        s_v = sct[n0:n1, 128:].rearrange("b (f d) -> b f d", d=3)
        for ddi in range(3):
            tpd = pp_misc.tile([nh, C], F32, tag="misc")
            nc.tensor.transpose(tpd[:], l1v[:, :, ddi], ident[:])
            nc.vector.tensor_add(o_v[:, :, ddi], tpd[:], s_v[:, :, ddi])
        nc.sync.dma_start(OUT[n0:n1], outt[:])


_PROG = {}


def kernel(**inputs):
    import concourse.bass_utils as bass_utils

    consts = _build_consts(inputs)

    nf = np.asarray(inputs["node_feats"], np.float32)
    attrs = np.asarray(inputs["node_attrs"], np.float32)
    sc = np.asarray(inputs["sc"], np.float32)

    if "prog" not in _PROG:
        _PROG["prog"] = build_program()
    nc = _PROG["prog"]

    in_maps = []
    for r in range(NCORES):
        b0 = r * BLOC
        xt = nf[b0:b0 + BLOC].transpose(2, 0, 1).reshape(16, NLOC)
        # 3-lane pack: lane Lb at partition base 32*Lb holds column blocks
        # [Lb*LBLK, (Lb+1)*LBLK)
        x3 = np.zeros((80, LANEW), np.float32)
        for blk in range(NBLK):
            Lb, cb = blk // LBLK, blk % LBLK
            x3[32 * Lb:32 * Lb + 16, cb * NB:(cb + 1) * NB] = xt[:, blk * NB:(blk + 1) * NB]
        m = {"X_Tm": x3,
             "attrsT": np.ascontiguousarray(attrs[b0:b0 + BLOC].T),
             "sc": np.ascontiguousarray(sc[b0:b0 + BLOC])}
        m.update(consts)
        in_maps.append(m)

    res = bass_utils.run_bass_kernel_spmd(
        nc, in_maps, list(range(NCORES)),
        trace=os.environ.get("KTRACE", "0") == "1")
    global LAST_EXEC_NS
    LAST_EXEC_NS = getattr(res, "exec_time_ns", None)
    outs = [np.asarray(res.results[r]["OUT"]) for r in range(NCORES)]
    return np.concatenate(outs, axis=0).astype(np.float32)


LAST_EXEC_NS = None
